# revision 22
# baseline (speedup 1.0000x reference)
"""DLRM forward (nn_DLRM_Net_498216206942) on 8 Trainium2 NeuronCores.

Sharding: data-parallel over the batch — each core takes 2048 of the 16384
samples, with the 26 embedding tables (bf16) and both MLPs replicated.

Per-core kernel layout (v3 — Gram-matmul interaction):
  - Bottom/top MLPs feature-major (features on partitions, batch on free dim).
  - Embedding lookup: one indirect DMA per 128-bag tile gathers all
    26 tables x 4 slots; pooling = 3 DVE/Pool adds (bf16).
  - Pooled features transposed to d-major via 13 two-feature [128,128] PE
    transposes per tile; PSUM halves copied (partition-shifted) into
    Tdm [64 d, 128 samples x 27 features] (feature 0 = bottom-MLP output).
  - Dot interaction: ONE Gram matmul per 4 samples:
    lhsT = rhs = Tdm[:, 108g:108g+108] (K=64, M=113 incl. pad, N=108)
    -> PSUM [113, 108]; diagonal 27x27 blocks are the per-sample Z.
  - Z scattered to zsb [64, 13 x 512] (j on partitions, 2 i-parities) by
    strided PSUM->SBUF copies on Vector/Scalar/Pool engines.
  - Top-MLP layer 0 consumes Z via symmetry: y += w0z_i^T @ zsb blocks,
    accumulated with the dense-x part in one PSUM group.
"""

import sys

sys.path.insert(0, "/opt/trn_rl_repo")

import numpy as np
import ml_dtypes

import concourse.bacc as bacc
import concourse.tile as tile
import concourse.mybir as mybir
from concourse.bass import IndirectOffsetOnAxis
from concourse.bass_utils import run_bass_kernel_spmd
from concourse.masks import make_identity

F32 = mybir.dt.float32
BF16 = mybir.dt.bfloat16
I32 = mybir.dt.int32

N_CORES = 8
N_TABLES = 26
VOCAB = 100000
D = 64
B = 16384
L = 4
BL = B // N_CORES          # 2048 samples per core
NF = N_TABLES + 1          # 27 features in T

_NC = None
LAST_RESULT = None
RUN_KWARGS = {}


def _build_nc():
    ntiles = BL // 128
    V = N_TABLES * VOCAB

    nc = bacc.Bacc("TRN2", target_bir_lowering=False, debug=False,
                   num_devices=N_CORES)

    emb = nc.dram_tensor("emb", [V, D], BF16, kind="ExternalInput")
    idx = nc.dram_tensor("idx", [BL, N_TABLES * L], I32, kind="ExternalInput")
    xt = nc.dram_tensor("xt", [13, BL], BF16, kind="ExternalInput")
    bw0 = nc.dram_tensor("bw0", [13, 512], BF16, kind="ExternalInput")
    bw1 = nc.dram_tensor("bw1", [128, 4 * 256], BF16, kind="ExternalInput")
    bw2 = nc.dram_tensor("bw2", [128, 2 * 64], BF16, kind="ExternalInput")
    bb0 = nc.dram_tensor("bb0", [128, 4], F32, kind="ExternalInput")
    bb1 = nc.dram_tensor("bb1", [128, 2], F32, kind="ExternalInput")
    bb2 = nc.dram_tensor("bb2", [64, 1], F32, kind="ExternalInput")
    # top layer 0: dense-x part [64, 512] bf16 + Z-row weights [128, 7*512]
    # (4 Z-rows packed per 128 partitions: row 32p+j = W0 weight of pair
    #  (i=4*gi+1+p, j))
    tw0x = nc.dram_tensor("tw0x", [64, 512], BF16, kind="ExternalInput")
    w0z = nc.dram_tensor("w0z", [128, 7 * 512], BF16, kind="ExternalInput")
    tw1 = nc.dram_tensor("tw1", [128, 4 * 256], BF16, kind="ExternalInput")
    tw2 = nc.dram_tensor("tw2", [128, 2], BF16, kind="ExternalInput")
    tb0 = nc.dram_tensor("tb0", [128, 4], F32, kind="ExternalInput")
    tb1 = nc.dram_tensor("tb1", [128, 2], F32, kind="ExternalInput")
    tb2 = nc.dram_tensor("tb2", [1, 1], F32, kind="ExternalInput")
    out = nc.dram_tensor("out", [BL], F32, kind="ExternalOutput")

    Relu = mybir.ActivationFunctionType.Relu
    Sigm = mybir.ActivationFunctionType.Sigmoid
    ADD = mybir.AluOpType.add

    bw_ = min(512, BL)
    nblk = BL // bw_

    with tile.TileContext(nc) as tc:
        with (
            tc.tile_pool(name="persist", bufs=1) as pp,
            tc.tile_pool(name="gather", bufs=3) as gpool,
            tc.tile_pool(name="idxp", bufs=6) as ipool,
            tc.tile_pool(name="gsum", bufs=3) as spool,
            tc.tile_pool(name="tdm", bufs=2) as dpool,
            tc.tile_pool(name="zsb", bufs=2) as zbpool,
            tc.tile_pool(name="psum_mm", bufs=3, space="PSUM") as pmm,
            tc.tile_pool(name="psum_tr", bufs=2, space="PSUM") as ptr,
            tc.tile_pool(name="psum_z", bufs=3, space="PSUM") as pzz,
        ):
            idx_tiles = []
            def emit_idx(bt):
                idx_sb = ipool.tile([128, N_TABLES * L], I32, tag="idx_sb",
                                    name="idx_sb")
                idx_tiles.append(idx_sb)
                nc.sync.dma_start(out=idx_sb[:],
                                  in_=idx[128 * bt:128 * (bt + 1), :])
            for bt in range(4):
                emit_idx(bt)

            def load(name, dram, shape, dtype=F32):
                t = pp.tile(shape, dtype, tag=name, name=name)
                nc.sync.dma_start(out=t[:], in_=dram[:])
                return t

            xt_sb = load("xt", xt, [13, BL], BF16)
            bw0_sb = load("bw0", bw0, [13, 512], BF16)
            bw1_sb = load("bw1", bw1, [128, 1024], BF16)
            bw2_sb = load("bw2", bw2, [128, 128], BF16)
            bb0_sb = load("bb0", bb0, [128, 4])
            bb1_sb = load("bb1", bb1, [128, 2])
            bb2_sb = load("bb2", bb2, [64, 1])
            tw0x_sb = load("tw0x", tw0x, [64, 512], BF16)
            w0z_sb = load("w0z", w0z, [128, 7 * 512], BF16)
            tw1_sb = load("tw1", tw1, [128, 1024], BF16)
            tw2_sb = load("tw2", tw2, [128, 2], BF16)
            tb0_sb = load("tb0", tb0, [128, 4])
            tb1_sb = load("tb1", tb1, [128, 2])
            tb2_sb = load("tb2", tb2, [1, 1])

            ident = pp.tile([128, 128], BF16, tag="ident", name="ident")
            make_identity(nc, ident[:])
            for bt in range(4, ntiles):
                emit_idx(bt)

            h1b = [[pp.tile([128, bw_], BF16, tag=f"h1_{m}_{p}",
                            name=f"h1_{m}_{p}") for m in range(4)]
                   for p in range(2)]
            h2b = [[pp.tile([128, bw_], BF16, tag=f"h2_{m}_{p}",
                            name=f"h2_{m}_{p}") for m in range(2)]
                   for p in range(2)]
            g1b = [[pp.tile([128, bw_], BF16, tag=f"g1_{m}_{p}",
                            name=f"g1_{m}_{p}") for m in range(4)]
                   for p in range(2)]
            g2b = [[pp.tile([128, bw_], BF16, tag=f"g2_{m}_{p}",
                            name=f"g2_{m}_{p}") for m in range(2)]
                   for p in range(2)]

            # ---- gather pipeline (gpsimd indirect DMA, slot-major order).
            # One DVE add folds 4 bag slots to 2; the final 2-way sum happens
            # in the transpose matmuls via PSUM accumulation ----
            gs_tiles = []
            SLOT = N_TABLES * D
            for bt in range(ntiles):
                idx_sb = idx_tiles[bt]
                g = gpool.tile([128, N_TABLES * L * D], BF16, tag="g", name="g")
                nc.gpsimd.indirect_dma_start(
                    out=g[:], out_offset=None,
                    in_=emb[:],
                    in_offset=IndirectOffsetOnAxis(ap=idx_sb[:], axis=0))
                gsum = spool.tile([128, 2 * SLOT], BF16, tag="gsum",
                                  name="gsum")
                gs_tiles.append(gsum)
                gvv = g[:].rearrange("p (a b c) -> p a b c", a=2, b=2)
                sv = gsum[:].rearrange("p (a c) -> p a c", a=2)
                nc.vector.tensor_tensor(sv, gvv[:, :, 0, :], gvv[:, :, 1, :],
                                        op=ADD)

            # ---- main loop: per chunk of 512 samples ----
            for nb in range(nblk):
                h1, h2 = h1b[nb % 2], h2b[nb % 2]
                g1, g2 = g1b[nb % 2], g2b[nb % 2]
                # bottom MLP (feature-major)
                for m in range(4):
                    ps = pmm.tile([128, bw_], F32, tag="ps", name="ps")
                    nc.tensor.matmul(ps[:], lhsT=bw0_sb[:, 128 * m:128 * (m + 1)],
                                     rhs=xt_sb[:, bw_ * nb:bw_ * (nb + 1)],
                                     start=True, stop=True)
                    nc.scalar.activation(h1[m][:], ps[:],
                                         Relu, bias=bb0_sb[:, m:m + 1])
                for m in range(2):
                    ps = pmm.tile([128, bw_], F32, tag="ps", name="ps")
                    for k in range(4):
                        nc.tensor.matmul(
                            ps[:],
                            lhsT=bw1_sb[:, 256 * k + 128 * m:256 * k + 128 * (m + 1)],
                            rhs=h1[k][:],
                            start=(k == 0), stop=(k == 3))
                    nc.scalar.activation(h2[m][:], ps[:],
                                         Relu, bias=bb1_sb[:, m:m + 1])
                x3 = pp.tile([64, bw_], BF16, tag=f"x3_{nb % 2}",
                             name=f"x3_{nb % 2}")
                ps = pmm.tile([64, bw_], F32, tag="ps", name="ps")
                for k in range(2):
                    nc.tensor.matmul(ps[:], lhsT=bw2_sb[:, 64 * k:64 * (k + 1)],
                                     rhs=h2[k][:],
                                     start=(k == 0), stop=(k == 1))
                nc.scalar.activation(x3[:], ps[:], Relu, bias=bb2_sb[:, 0:1])

                zsb = zbpool.tile([128, 7 * bw_], BF16, tag="zsb", name="zsb")
                zsv = zsb[:].rearrange("p (gi s) -> p gi s", gi=7)

                for bq in range(4):
                    bt = 4 * nb + bq
                    gsum = gs_tiles[bt]  # [128, 2 slot-pairs x 26 x 64]
                    # Tdm [64 d, 32 features x 128 samples] feature-major so
                    # all copy APs have contiguous inner runs; features 27:32
                    # are pads (zeroed once per buffer)
                    tdm = dpool.tile([64, 32 * 128], BF16, tag="tdm",
                                     name="tdm")
                    tdmF = tdm[:].rearrange("p (f s) -> p f s", s=128)
                    if nb == 0 and bq < 2:
                        nc.vector.memset(tdmF[:, NF:32, :], 0.0)
                    # feature 0 = bottom-MLP output
                    nc.gpsimd.tensor_copy(
                        tdmF[:, 0:1, :],
                        x3[:, 128 * bq:128 * (bq + 1)].rearrange(
                            "p (a s) -> p a s", a=1))
                    # features 1..26: two-feature [128,128] PE transposes; the
                    # 2 slot-pairs accumulate in PSUM (rest of the pooling)
                    for w in range(4):
                        p0 = 4 * w
                        npair = min(4, 13 - p0)
                        pst = ptr.tile([128, 512], F32, tag="pst", name="pst")
                        for t in range(npair):
                            blk = 2 * (p0 + t)
                            for sp in range(2):
                                c0 = SLOT * sp + 64 * blk
                                nc.tensor.matmul(
                                    pst[:, 128 * t:128 * (t + 1)],
                                    lhsT=gsum[:, c0:c0 + 128],
                                    rhs=ident[:], start=(sp == 0),
                                    stop=(sp == 1))
                        pstv = pst[:].rearrange("p (t s) -> p t s", s=128)
                        f1 = 2 * p0 + 1
                        nc.vector.tensor_copy(
                            tdmF[0:64, f1:f1 + 2 * npair - 1:2, :],
                            pstv[0:64, 0:npair, :])
                        nc.scalar.copy(
                            tdmF[0:64, f1 + 1:f1 + 2 * npair:2, :],
                            pstv[64:128, 0:npair, :])

                    # interaction: one [64,32]x[64,26] matmul per sample ->
                    # Z columns land with j uniformly on partitions 0:32;
                    # extraction = 2 batched copies per 16-sample PSUM bank
                    for sw in range(8):
                        zp = pzz.tile([32, 416], F32, tag="zp", name="zp")
                        for sl in range(16):
                            ss = 16 * sw + sl
                            nc.tensor.matmul(
                                zp[:, 26 * sl:26 * (sl + 1)],
                                lhsT=tdm[:, ss:ss + 31 * 128 + 1:128],
                                rhs=tdm[:, 128 + ss:128 + ss + 25 * 128 + 1:128],
                                start=True, stop=True)
                        zpv = zp[:].rearrange("p (s i) -> p i s", i=26)
                        s0 = 128 * bq + 16 * sw
                        for r in range(4):
                            ngi = 7 if r < 2 else 6
                            src = zpv[:, r:r + 4 * (ngi - 1) + 1:4, :]
                            dst = zsv[32 * r:32 * (r + 1), 0:ngi, s0:s0 + 16]
                            if r % 2 == 0:
                                nc.vector.tensor_copy(dst, src)
                            else:
                                nc.scalar.copy(dst, src)

                # ---- top MLP layer 0: dense-x part + Z via symmetry ----
                for m in range(4):
                    ps = pmm.tile([128, bw_], F32, tag="ps", name="ps")
                    nc.tensor.matmul(ps[:],
                                     lhsT=tw0x_sb[:, 128 * m:128 * (m + 1)],
                                     rhs=x3[:],
                                     start=True, stop=False)
                    for gi in range(7):
                        nc.tensor.matmul(
                            ps[:],
                            lhsT=w0z_sb[:,
                                        512 * gi + 128 * m:
                                        512 * gi + 128 * (m + 1)],
                            rhs=zsb[:, bw_ * gi:bw_ * (gi + 1)],
                            start=False, stop=(gi == 6))
                    nc.scalar.activation(g1[m][:], ps[:],
                                         Relu, bias=tb0_sb[:, m:m + 1])
                # layers 1, 2
                for m in range(2):
                    ps = pmm.tile([128, bw_], F32, tag="ps", name="ps")
                    for k in range(4):
                        nc.tensor.matmul(
                            ps[:],
                            lhsT=tw1_sb[:, 256 * k + 128 * m:
                                        256 * k + 128 * (m + 1)],
                            rhs=g1[k][:],
                            start=(k == 0), stop=(k == 3))
                    nc.scalar.activation(g2[m][:], ps[:],
                                         Relu, bias=tb1_sb[:, m:m + 1])
                zo = pp.tile([1, bw_], F32, tag=f"zo_{nb}", name=f"zo_{nb}")
                ps = pmm.tile([1, bw_], F32, tag="ps", name="ps")
                for k in range(2):
                    nc.tensor.matmul(ps[:], lhsT=tw2_sb[:, k:k + 1],
                                     rhs=g2[k][:],
                                     start=(k == 0), stop=(k == 1))
                nc.scalar.activation(zo[:], ps[:], Sigm, bias=tb2_sb[:, 0:1])
                nc.sync.dma_start(out=out[bw_ * nb:bw_ * (nb + 1)], in_=zo[:])

    nc.compile()
    return nc


def _get_nc():
    global _NC
    if _NC is None:
        _NC = _build_nc()
    return _NC


def kernel(**inputs) -> np.ndarray:
    global LAST_RESULT
    nc = _get_nc()

    emb_bf = np.ascontiguousarray(
        np.asarray(inputs["emb"], dtype=np.float32).reshape(N_TABLES * VOCAB, D)
    ).astype(ml_dtypes.bfloat16)

    dense_x = np.asarray(inputs["dense_x"], dtype=np.float32)
    lS_i = np.asarray(inputs["lS_i"]).reshape(N_TABLES, B, L)
    table_base = np.arange(N_TABLES, dtype=np.int64)[:, None, None] * VOCAB

    def kt(w, p=128):  # [K, M] -> [p, (K//p)*M], k-tiles side by side
        K, M = w.shape
        return np.ascontiguousarray(
            w.reshape(K // p, p, M).transpose(1, 0, 2).reshape(p, -1))

    def bvec(b, p=128):  # [M] -> [p, M//p] (or [M, 1] when M < p)
        M = b.shape[0]
        if M < p:
            return np.ascontiguousarray(b.reshape(M, 1))
        return np.ascontiguousarray(b.reshape(M // p, p).T)

    W = {k: np.asarray(v, dtype=np.float32) for k, v in inputs.items()
         if k.startswith(("bot_", "top_"))}
    t0 = W["top_W0"].T  # [415, 512]

    # w0z[32*r + j, 512*gi + m] = W0 weight of pair (i=4*gi+1+r, j), j < i.
    # Reference pair order: (i, j) for i in range(27) for j in range(i),
    # pair index p = i*(i-1)/2 + j; R = [x (64 cols), Zflat (351)].
    w0z_np = np.zeros((128, 7 * 512), dtype=np.float32)
    zw = t0[64:]  # [351, 512]
    for gi in range(7):
        for r in range(4):
            i = 4 * gi + 1 + r
            if i > 26:
                continue
            base = i * (i - 1) // 2
            w0z_np[32 * r:32 * r + i, 512 * gi:512 * (gi + 1)] = zw[base:base + i]

    shared = {
        "emb": emb_bf,
        "bw0": np.ascontiguousarray(W["bot_W0"].T).astype(ml_dtypes.bfloat16),
        "bw1": kt(W["bot_W1"].T).astype(ml_dtypes.bfloat16),
        "bw2": kt(W["bot_W2"].T).astype(ml_dtypes.bfloat16),
        "bb0": bvec(W["bot_b0"]),
        "bb1": bvec(W["bot_b1"]),
        "bb2": bvec(W["bot_b2"]),
        "tw0x": np.ascontiguousarray(t0[:64]).astype(ml_dtypes.bfloat16),
        "w0z": w0z_np.astype(ml_dtypes.bfloat16),
        "tw1": kt(W["top_W1"].T).astype(ml_dtypes.bfloat16),
        "tw2": kt(W["top_W2"].T).astype(ml_dtypes.bfloat16),
        "tb0": bvec(W["top_b0"]),
        "tb1": bvec(W["top_b1"]),
        "tb2": bvec(W["top_b2"]),
    }

    in_maps = []
    for c in range(N_CORES):
        b0 = c * BL
        # slot-major gather order: idx cols = [slot, table]
        idxc = (table_base + lS_i[:, b0:b0 + BL, :]).transpose(1, 2, 0)
        in_maps.append(dict(
            shared,
            idx=np.ascontiguousarray(idxc.reshape(BL, N_TABLES * L)).astype(np.int32),
            xt=np.ascontiguousarray(dense_x[b0:b0 + BL].T).astype(ml_dtypes.bfloat16),
        ))

    res = run_bass_kernel_spmd(nc, in_maps, core_ids=list(range(N_CORES)),
                               **RUN_KWARGS)
    LAST_RESULT = res
    out = np.concatenate([np.asarray(res.results[c]["out"]) for c in range(N_CORES)])
    return out.reshape(B, 1).astype(np.float32)


# revision 23
# speedup vs baseline: 1.0355x; 1.0355x over previous
"""DLRM forward (nn_DLRM_Net_498216206942) on 8 Trainium2 NeuronCores.

Sharding: data-parallel over the batch — each core takes 2048 of the 16384
samples, with the 26 embedding tables (bf16) and both MLPs replicated.

Per-core kernel layout (v3 — Gram-matmul interaction):
  - Bottom/top MLPs feature-major (features on partitions, batch on free dim).
  - Embedding lookup: one indirect DMA per 128-bag tile gathers all
    26 tables x 4 slots; pooling = 3 DVE/Pool adds (bf16).
  - Pooled features transposed to d-major via 13 two-feature [128,128] PE
    transposes per tile; PSUM halves copied (partition-shifted) into
    Tdm [64 d, 128 samples x 27 features] (feature 0 = bottom-MLP output).
  - Dot interaction: ONE Gram matmul per 4 samples:
    lhsT = rhs = Tdm[:, 108g:108g+108] (K=64, M=113 incl. pad, N=108)
    -> PSUM [113, 108]; diagonal 27x27 blocks are the per-sample Z.
  - Z scattered to zsb [64, 13 x 512] (j on partitions, 2 i-parities) by
    strided PSUM->SBUF copies on Vector/Scalar/Pool engines.
  - Top-MLP layer 0 consumes Z via symmetry: y += w0z_i^T @ zsb blocks,
    accumulated with the dense-x part in one PSUM group.
"""

import sys

sys.path.insert(0, "/opt/trn_rl_repo")

import numpy as np
import ml_dtypes

import concourse.bacc as bacc
import concourse.tile as tile
import concourse.mybir as mybir
from concourse.bass import IndirectOffsetOnAxis
from concourse.bass_utils import run_bass_kernel_spmd
from concourse.masks import make_identity

F32 = mybir.dt.float32
BF16 = mybir.dt.bfloat16
I32 = mybir.dt.int32

N_CORES = 8
N_TABLES = 26
VOCAB = 100000
D = 64
B = 16384
L = 4
BL = B // N_CORES          # 2048 samples per core
NF = N_TABLES + 1          # 27 features in T

_NC = None
LAST_RESULT = None
RUN_KWARGS = {}


def _build_nc():
    ntiles = BL // 128
    V = N_TABLES * VOCAB

    nc = bacc.Bacc("TRN2", target_bir_lowering=False, debug=False,
                   num_devices=N_CORES)

    emb = nc.dram_tensor("emb", [V, D], BF16, kind="ExternalInput")
    idx = nc.dram_tensor("idx", [BL, N_TABLES * L], I32, kind="ExternalInput")
    xt = nc.dram_tensor("xt", [13, BL], BF16, kind="ExternalInput")
    bw0 = nc.dram_tensor("bw0", [13, 512], BF16, kind="ExternalInput")
    bw1 = nc.dram_tensor("bw1", [128, 4 * 256], BF16, kind="ExternalInput")
    bw2 = nc.dram_tensor("bw2", [128, 2 * 64], BF16, kind="ExternalInput")
    bb0 = nc.dram_tensor("bb0", [128, 4], F32, kind="ExternalInput")
    bb1 = nc.dram_tensor("bb1", [128, 2], F32, kind="ExternalInput")
    bb2 = nc.dram_tensor("bb2", [64, 1], F32, kind="ExternalInput")
    # top layer 0: dense-x part [64, 512] bf16 + Z-row weights [64, 13*512]
    tw0x = nc.dram_tensor("tw0x", [64, 512], BF16, kind="ExternalInput")
    w0z = nc.dram_tensor("w0z", [64, 13 * 512], BF16, kind="ExternalInput")
    tw1 = nc.dram_tensor("tw1", [128, 4 * 256], BF16, kind="ExternalInput")
    tw2 = nc.dram_tensor("tw2", [128, 2], BF16, kind="ExternalInput")
    tb0 = nc.dram_tensor("tb0", [128, 4], F32, kind="ExternalInput")
    tb1 = nc.dram_tensor("tb1", [128, 2], F32, kind="ExternalInput")
    tb2 = nc.dram_tensor("tb2", [1, 1], F32, kind="ExternalInput")
    out = nc.dram_tensor("out", [BL], F32, kind="ExternalOutput")

    Relu = mybir.ActivationFunctionType.Relu
    Sigm = mybir.ActivationFunctionType.Sigmoid
    ADD = mybir.AluOpType.add

    bw_ = min(512, BL)
    nblk = BL // bw_

    with tile.TileContext(nc) as tc:
        with (
            tc.tile_pool(name="persist", bufs=1) as pp,
            tc.tile_pool(name="gather", bufs=3) as gpool,
            tc.tile_pool(name="idxp", bufs=6) as ipool,
            tc.tile_pool(name="gsum", bufs=3) as spool,
            tc.tile_pool(name="tdm", bufs=2) as dpool,
            tc.tile_pool(name="zsb", bufs=2) as zbpool,
            tc.tile_pool(name="psum_mm", bufs=3, space="PSUM") as pmm,
            tc.tile_pool(name="psum_tr", bufs=2, space="PSUM") as ptr,
            tc.tile_pool(name="psum_z", bufs=3, space="PSUM") as pzz,
        ):
            idx_tiles = []
            def emit_idx(bt):
                idx_sb = ipool.tile([128, N_TABLES * L], I32, tag="idx_sb",
                                    name="idx_sb")
                idx_tiles.append(idx_sb)
                nc.sync.dma_start(out=idx_sb[:],
                                  in_=idx[128 * bt:128 * (bt + 1), :])
            for bt in range(4):
                emit_idx(bt)

            def load(name, dram, shape, dtype=F32):
                t = pp.tile(shape, dtype, tag=name, name=name)
                nc.sync.dma_start(out=t[:], in_=dram[:])
                return t

            xt_sb = load("xt", xt, [13, BL], BF16)
            bw0_sb = load("bw0", bw0, [13, 512], BF16)
            bw1_sb = load("bw1", bw1, [128, 1024], BF16)
            bw2_sb = load("bw2", bw2, [128, 128], BF16)
            bb0_sb = load("bb0", bb0, [128, 4])
            bb1_sb = load("bb1", bb1, [128, 2])
            bb2_sb = load("bb2", bb2, [64, 1])
            tw0x_sb = load("tw0x", tw0x, [64, 512], BF16)
            w0z_sb = load("w0z", w0z, [64, 13 * 512], BF16)
            tw1_sb = load("tw1", tw1, [128, 1024], BF16)
            tw2_sb = load("tw2", tw2, [128, 2], BF16)
            tb0_sb = load("tb0", tb0, [128, 4])
            tb1_sb = load("tb1", tb1, [128, 2])
            tb2_sb = load("tb2", tb2, [1, 1])

            ident = pp.tile([128, 128], BF16, tag="ident", name="ident")
            make_identity(nc, ident[:])
            for bt in range(4, ntiles):
                emit_idx(bt)

            h1b = [[pp.tile([128, bw_], BF16, tag=f"h1_{m}_{p}",
                            name=f"h1_{m}_{p}") for m in range(4)]
                   for p in range(2)]
            h2b = [[pp.tile([128, bw_], BF16, tag=f"h2_{m}_{p}",
                            name=f"h2_{m}_{p}") for m in range(2)]
                   for p in range(2)]
            g1b = [[pp.tile([128, bw_], BF16, tag=f"g1_{m}_{p}",
                            name=f"g1_{m}_{p}") for m in range(4)]
                   for p in range(2)]
            g2b = [[pp.tile([128, bw_], BF16, tag=f"g2_{m}_{p}",
                            name=f"g2_{m}_{p}") for m in range(2)]
                   for p in range(2)]

            # ---- gather pipeline (gpsimd indirect DMA, slot-major order).
            # One DVE add folds 4 bag slots to 2; the final 2-way sum happens
            # in the transpose matmuls via PSUM accumulation ----
            gs_tiles = []
            SLOT = N_TABLES * D
            for bt in range(ntiles):
                idx_sb = idx_tiles[bt]
                g = gpool.tile([128, N_TABLES * L * D], BF16, tag="g", name="g")
                nc.gpsimd.indirect_dma_start(
                    out=g[:], out_offset=None,
                    in_=emb[:],
                    in_offset=IndirectOffsetOnAxis(ap=idx_sb[:], axis=0))
                gsum = spool.tile([128, 2 * SLOT], BF16, tag="gsum",
                                  name="gsum")
                gs_tiles.append(gsum)
                gvv = g[:].rearrange("p (a b c) -> p a b c", a=2, b=2)
                sv = gsum[:].rearrange("p (a c) -> p a c", a=2)
                nc.vector.tensor_tensor(sv, gvv[:, :, 0, :], gvv[:, :, 1, :],
                                        op=ADD)

            # ---- main loop: per chunk of 512 samples ----
            for nb in range(nblk):
                h1, h2 = h1b[nb % 2], h2b[nb % 2]
                g1, g2 = g1b[nb % 2], g2b[nb % 2]
                # bottom MLP (feature-major)
                for m in range(4):
                    ps = pmm.tile([128, bw_], F32, tag="ps", name="ps")
                    nc.tensor.matmul(ps[:], lhsT=bw0_sb[:, 128 * m:128 * (m + 1)],
                                     rhs=xt_sb[:, bw_ * nb:bw_ * (nb + 1)],
                                     start=True, stop=True)
                    nc.scalar.activation(h1[m][:], ps[:],
                                         Relu, bias=bb0_sb[:, m:m + 1])
                for m in range(2):
                    ps = pmm.tile([128, bw_], F32, tag="ps", name="ps")
                    for k in range(4):
                        nc.tensor.matmul(
                            ps[:],
                            lhsT=bw1_sb[:, 256 * k + 128 * m:256 * k + 128 * (m + 1)],
                            rhs=h1[k][:],
                            start=(k == 0), stop=(k == 3))
                    nc.scalar.activation(h2[m][:], ps[:],
                                         Relu, bias=bb1_sb[:, m:m + 1])
                x3 = pp.tile([64, bw_], BF16, tag=f"x3_{nb % 2}",
                             name=f"x3_{nb % 2}")
                ps = pmm.tile([64, bw_], F32, tag="ps", name="ps")
                for k in range(2):
                    nc.tensor.matmul(ps[:], lhsT=bw2_sb[:, 64 * k:64 * (k + 1)],
                                     rhs=h2[k][:],
                                     start=(k == 0), stop=(k == 1))
                nc.scalar.activation(x3[:], ps[:], Relu, bias=bb2_sb[:, 0:1])

                zsb = zbpool.tile([64, 13 * bw_], BF16, tag="zsb", name="zsb")
                zsv = zsb[:].rearrange("p (gi s) -> p gi s", gi=13)

                for bq in range(4):
                    bt = 4 * nb + bq
                    gsum = gs_tiles[bt]  # [128, 2 slot-pairs x 26 x 64]
                    # Tdm [64 d, 32 features x 128 samples] feature-major so
                    # all copy APs have contiguous inner runs; features 27:32
                    # are pads (zeroed once per buffer)
                    tdm = dpool.tile([64, 32 * 128], BF16, tag="tdm",
                                     name="tdm")
                    tdmF = tdm[:].rearrange("p (f s) -> p f s", s=128)
                    if nb == 0 and bq < 2:
                        nc.vector.memset(tdmF[:, NF:32, :], 0.0)
                    # feature 0 = bottom-MLP output
                    nc.gpsimd.tensor_copy(
                        tdmF[:, 0:1, :],
                        x3[:, 128 * bq:128 * (bq + 1)].rearrange(
                            "p (a s) -> p a s", a=1))
                    # features 1..26: two-feature [128,128] PE transposes; the
                    # 2 slot-pairs accumulate in PSUM (rest of the pooling)
                    for w in range(4):
                        p0 = 4 * w
                        npair = min(4, 13 - p0)
                        pst = ptr.tile([128, 512], F32, tag="pst", name="pst")
                        for t in range(npair):
                            blk = 2 * (p0 + t)
                            for sp in range(2):
                                c0 = SLOT * sp + 64 * blk
                                nc.tensor.matmul(
                                    pst[:, 128 * t:128 * (t + 1)],
                                    lhsT=gsum[:, c0:c0 + 128],
                                    rhs=ident[:], start=(sp == 0),
                                    stop=(sp == 1))
                        pstv = pst[:].rearrange("p (t s) -> p t s", s=128)
                        f1 = 2 * p0 + 1
                        nc.vector.tensor_copy(
                            tdmF[0:64, f1:f1 + 2 * npair - 1:2, :],
                            pstv[0:64, 0:npair, :])
                        nc.scalar.copy(
                            tdmF[0:64, f1 + 1:f1 + 2 * npair:2, :],
                            pstv[64:128, 0:npair, :])

                    # interaction: one [64,32]x[64,26] matmul per sample ->
                    # Z columns land with j uniformly on partitions 0:32;
                    # extraction = 2 batched copies per 16-sample PSUM bank
                    for sw in range(8):
                        zp = pzz.tile([32, 416], F32, tag="zp", name="zp")
                        for sl in range(16):
                            ss = 16 * sw + sl
                            nc.tensor.matmul(
                                zp[:, 26 * sl:26 * (sl + 1)],
                                lhsT=tdm[:, ss:ss + 31 * 128 + 1:128],
                                rhs=tdm[:, 128 + ss:128 + ss + 25 * 128 + 1:128],
                                start=True, stop=True)
                        zpv = zp[:].rearrange("p (s i) -> p i s", i=26)
                        s0 = 128 * bq + 16 * sw
                        for r in range(2):
                            src = zpv[:, r:r + 25:2, :]
                            dst = zsv[32 * r:32 * (r + 1), :, s0:s0 + 16]
                            if r == 0:
                                nc.vector.tensor_copy(dst, src)
                            else:
                                nc.scalar.copy(dst, src)

                # ---- top MLP layer 0: dense-x part + Z via symmetry ----
                for m in range(4):
                    ps = pmm.tile([128, bw_], F32, tag="ps", name="ps")
                    nc.tensor.matmul(ps[:],
                                     lhsT=tw0x_sb[:, 128 * m:128 * (m + 1)],
                                     rhs=x3[:],
                                     start=True, stop=False)
                    for gi in range(13):
                        nc.tensor.matmul(
                            ps[:],
                            lhsT=w0z_sb[:,
                                        512 * gi + 128 * m:
                                        512 * gi + 128 * (m + 1)],
                            rhs=zsb[:, bw_ * gi:bw_ * (gi + 1)],
                            start=False, stop=(gi == 12))
                    nc.scalar.activation(g1[m][:], ps[:],
                                         Relu, bias=tb0_sb[:, m:m + 1])
                # layers 1, 2
                for m in range(2):
                    ps = pmm.tile([128, bw_], F32, tag="ps", name="ps")
                    for k in range(4):
                        nc.tensor.matmul(
                            ps[:],
                            lhsT=tw1_sb[:, 256 * k + 128 * m:
                                        256 * k + 128 * (m + 1)],
                            rhs=g1[k][:],
                            start=(k == 0), stop=(k == 3))
                    nc.scalar.activation(g2[m][:], ps[:],
                                         Relu, bias=tb1_sb[:, m:m + 1])
                zo = pp.tile([1, bw_], F32, tag=f"zo_{nb}", name=f"zo_{nb}")
                ps = pmm.tile([1, bw_], F32, tag="ps", name="ps")
                for k in range(2):
                    nc.tensor.matmul(ps[:], lhsT=tw2_sb[:, k:k + 1],
                                     rhs=g2[k][:],
                                     start=(k == 0), stop=(k == 1))
                nc.scalar.activation(zo[:], ps[:], Sigm, bias=tb2_sb[:, 0:1])
                nc.sync.dma_start(out=out[bw_ * nb:bw_ * (nb + 1)], in_=zo[:])

    nc.compile()
    return nc


def _get_nc():
    global _NC
    if _NC is None:
        _NC = _build_nc()
    return _NC


def kernel(**inputs) -> np.ndarray:
    global LAST_RESULT
    nc = _get_nc()

    emb_bf = np.ascontiguousarray(
        np.asarray(inputs["emb"], dtype=np.float32).reshape(N_TABLES * VOCAB, D)
    ).astype(ml_dtypes.bfloat16)

    dense_x = np.asarray(inputs["dense_x"], dtype=np.float32)
    lS_i = np.asarray(inputs["lS_i"]).reshape(N_TABLES, B, L)
    table_base = np.arange(N_TABLES, dtype=np.int64)[:, None, None] * VOCAB

    def kt(w, p=128):  # [K, M] -> [p, (K//p)*M], k-tiles side by side
        K, M = w.shape
        return np.ascontiguousarray(
            w.reshape(K // p, p, M).transpose(1, 0, 2).reshape(p, -1))

    def bvec(b, p=128):  # [M] -> [p, M//p] (or [M, 1] when M < p)
        M = b.shape[0]
        if M < p:
            return np.ascontiguousarray(b.reshape(M, 1))
        return np.ascontiguousarray(b.reshape(M // p, p).T)

    W = {k: np.asarray(v, dtype=np.float32) for k, v in inputs.items()
         if k.startswith(("bot_", "top_"))}
    t0 = W["top_W0"].T  # [415, 512]

    # w0z[32*r + j, 512*gi + m] = W0 weight of pair (i=2*gi+1+r, j), j < i.
    # Reference pair order: (i, j) for i in range(27) for j in range(i),
    # pair index p = i*(i-1)/2 + j; R = [x (64 cols), Zflat (351)].
    w0z_np = np.zeros((64, 13 * 512), dtype=np.float32)
    zw = t0[64:]  # [351, 512]
    for gi in range(13):
        for r in range(2):
            i = 2 * gi + 1 + r
            base = i * (i - 1) // 2
            w0z_np[32 * r:32 * r + i, 512 * gi:512 * (gi + 1)] = zw[base:base + i]

    shared = {
        "emb": emb_bf,
        "bw0": np.ascontiguousarray(W["bot_W0"].T).astype(ml_dtypes.bfloat16),
        "bw1": kt(W["bot_W1"].T).astype(ml_dtypes.bfloat16),
        "bw2": kt(W["bot_W2"].T).astype(ml_dtypes.bfloat16),
        "bb0": bvec(W["bot_b0"]),
        "bb1": bvec(W["bot_b1"]),
        "bb2": bvec(W["bot_b2"]),
        "tw0x": np.ascontiguousarray(t0[:64]).astype(ml_dtypes.bfloat16),
        "w0z": w0z_np.astype(ml_dtypes.bfloat16),
        "tw1": kt(W["top_W1"].T).astype(ml_dtypes.bfloat16),
        "tw2": kt(W["top_W2"].T).astype(ml_dtypes.bfloat16),
        "tb0": bvec(W["top_b0"]),
        "tb1": bvec(W["top_b1"]),
        "tb2": bvec(W["top_b2"]),
    }

    in_maps = []
    for c in range(N_CORES):
        b0 = c * BL
        # slot-major gather order: idx cols = [slot, table]
        idxc = (table_base + lS_i[:, b0:b0 + BL, :]).transpose(1, 2, 0)
        in_maps.append(dict(
            shared,
            idx=np.ascontiguousarray(idxc.reshape(BL, N_TABLES * L)).astype(np.int32),
            xt=np.ascontiguousarray(dense_x[b0:b0 + BL].T).astype(ml_dtypes.bfloat16),
        ))

    res = run_bass_kernel_spmd(nc, in_maps, core_ids=list(range(N_CORES)),
                               **RUN_KWARGS)
    LAST_RESULT = res
    out = np.concatenate([np.asarray(res.results[c]["out"]) for c in range(N_CORES)])
    return out.reshape(B, 1).astype(np.float32)


# revision 24
# speedup vs baseline: 1.0578x; 1.0216x over previous
"""DLRM forward (nn_DLRM_Net_498216206942) on 8 Trainium2 NeuronCores.

Sharding: data-parallel over the batch — each core takes 2048 of the 16384
samples, with the 26 embedding tables (bf16) and both MLPs replicated.

Per-core kernel layout (v3 — Gram-matmul interaction):
  - Bottom/top MLPs feature-major (features on partitions, batch on free dim).
  - Embedding lookup: one indirect DMA per 128-bag tile gathers all
    26 tables x 4 slots; pooling = 3 DVE/Pool adds (bf16).
  - Pooled features transposed to d-major via 13 two-feature [128,128] PE
    transposes per tile; PSUM halves copied (partition-shifted) into
    Tdm [64 d, 128 samples x 27 features] (feature 0 = bottom-MLP output).
  - Dot interaction: ONE Gram matmul per 4 samples:
    lhsT = rhs = Tdm[:, 108g:108g+108] (K=64, M=113 incl. pad, N=108)
    -> PSUM [113, 108]; diagonal 27x27 blocks are the per-sample Z.
  - Z scattered to zsb [64, 13 x 512] (j on partitions, 2 i-parities) by
    strided PSUM->SBUF copies on Vector/Scalar/Pool engines.
  - Top-MLP layer 0 consumes Z via symmetry: y += w0z_i^T @ zsb blocks,
    accumulated with the dense-x part in one PSUM group.
"""

import sys

sys.path.insert(0, "/opt/trn_rl_repo")

import numpy as np
import ml_dtypes

import concourse.bacc as bacc
import concourse.tile as tile
import concourse.mybir as mybir
from concourse.bass import IndirectOffsetOnAxis
from concourse.bass_utils import run_bass_kernel_spmd
from concourse.masks import make_identity

F32 = mybir.dt.float32
BF16 = mybir.dt.bfloat16
I32 = mybir.dt.int32

N_CORES = 8
N_TABLES = 26
VOCAB = 100000
D = 64
B = 16384
L = 4
BL = B // N_CORES          # 2048 samples per core
NF = N_TABLES + 1          # 27 features in T

_NC = None
LAST_RESULT = None
RUN_KWARGS = {}


def _build_nc():
    ntiles = BL // 128
    V = N_TABLES * VOCAB

    nc = bacc.Bacc("TRN2", target_bir_lowering=False, debug=False,
                   num_devices=N_CORES)

    emb = nc.dram_tensor("emb", [V, D], BF16, kind="ExternalInput")
    idx = nc.dram_tensor("idx", [BL, N_TABLES * L], I32, kind="ExternalInput")
    xt = nc.dram_tensor("xt", [13, BL], BF16, kind="ExternalInput")
    bw0 = nc.dram_tensor("bw0", [13, 512], BF16, kind="ExternalInput")
    bw1 = nc.dram_tensor("bw1", [128, 4 * 256], BF16, kind="ExternalInput")
    bw2 = nc.dram_tensor("bw2", [128, 2 * 64], BF16, kind="ExternalInput")
    bb0 = nc.dram_tensor("bb0", [128, 4], F32, kind="ExternalInput")
    bb1 = nc.dram_tensor("bb1", [128, 2], F32, kind="ExternalInput")
    bb2 = nc.dram_tensor("bb2", [64, 1], F32, kind="ExternalInput")
    # top layer 0: dense-x part [64, 512] bf16 + Z-row weights [64, 13*512]
    tw0x = nc.dram_tensor("tw0x", [64, 512], BF16, kind="ExternalInput")
    w0z = nc.dram_tensor("w0z", [64, 13 * 512], BF16, kind="ExternalInput")
    tw1 = nc.dram_tensor("tw1", [128, 4 * 256], BF16, kind="ExternalInput")
    tw2 = nc.dram_tensor("tw2", [128, 2], BF16, kind="ExternalInput")
    tb0 = nc.dram_tensor("tb0", [128, 4], F32, kind="ExternalInput")
    tb1 = nc.dram_tensor("tb1", [128, 2], F32, kind="ExternalInput")
    tb2 = nc.dram_tensor("tb2", [1, 1], F32, kind="ExternalInput")
    out = nc.dram_tensor("out", [BL], F32, kind="ExternalOutput")

    Relu = mybir.ActivationFunctionType.Relu
    Sigm = mybir.ActivationFunctionType.Sigmoid
    ADD = mybir.AluOpType.add

    bw_ = min(512, BL)
    nblk = BL // bw_

    with tile.TileContext(nc) as tc:
        with (
            tc.tile_pool(name="persist", bufs=1) as pp,
            tc.tile_pool(name="gather", bufs=3) as gpool,
            tc.tile_pool(name="idxp", bufs=6) as ipool,
            tc.tile_pool(name="gsum", bufs=3) as spool,
            tc.tile_pool(name="tdm", bufs=4) as dpool,
            tc.tile_pool(name="zsb", bufs=2) as zbpool,
            tc.tile_pool(name="psum_mm", bufs=3, space="PSUM") as pmm,
            tc.tile_pool(name="psum_tr", bufs=3, space="PSUM") as ptr,
            tc.tile_pool(name="psum_z", bufs=2, space="PSUM") as pzz,
        ):
            idx_tiles = []
            def emit_idx(bt):
                idx_sb = ipool.tile([128, N_TABLES * L], I32, tag="idx_sb",
                                    name="idx_sb")
                idx_tiles.append(idx_sb)
                nc.sync.dma_start(out=idx_sb[:],
                                  in_=idx[128 * bt:128 * (bt + 1), :])
            for bt in range(4):
                emit_idx(bt)

            def load(name, dram, shape, dtype=F32):
                t = pp.tile(shape, dtype, tag=name, name=name)
                nc.sync.dma_start(out=t[:], in_=dram[:])
                return t

            xt_sb = load("xt", xt, [13, BL], BF16)
            bw0_sb = load("bw0", bw0, [13, 512], BF16)
            bw1_sb = load("bw1", bw1, [128, 1024], BF16)
            bw2_sb = load("bw2", bw2, [128, 128], BF16)
            bb0_sb = load("bb0", bb0, [128, 4])
            bb1_sb = load("bb1", bb1, [128, 2])
            bb2_sb = load("bb2", bb2, [64, 1])
            tw0x_sb = load("tw0x", tw0x, [64, 512], BF16)
            w0z_sb = load("w0z", w0z, [64, 13 * 512], BF16)
            tw1_sb = load("tw1", tw1, [128, 1024], BF16)
            tw2_sb = load("tw2", tw2, [128, 2], BF16)
            tb0_sb = load("tb0", tb0, [128, 4])
            tb1_sb = load("tb1", tb1, [128, 2])
            tb2_sb = load("tb2", tb2, [1, 1])

            ident = pp.tile([128, 128], BF16, tag="ident", name="ident")
            make_identity(nc, ident[:])
            for bt in range(4, ntiles):
                emit_idx(bt)

            h1b = [[pp.tile([128, bw_], BF16, tag=f"h1_{m}_{p}",
                            name=f"h1_{m}_{p}") for m in range(4)]
                   for p in range(2)]
            h2b = [[pp.tile([128, bw_], BF16, tag=f"h2_{m}_{p}",
                            name=f"h2_{m}_{p}") for m in range(2)]
                   for p in range(2)]
            g1b = [[pp.tile([128, bw_], BF16, tag=f"g1_{m}_{p}",
                            name=f"g1_{m}_{p}") for m in range(4)]
                   for p in range(2)]
            g2b = [[pp.tile([128, bw_], BF16, tag=f"g2_{m}_{p}",
                            name=f"g2_{m}_{p}") for m in range(2)]
                   for p in range(2)]

            # ---- gather pipeline (gpsimd indirect DMA, slot-major order).
            # One DVE add folds 4 bag slots to 2; the final 2-way sum happens
            # in the transpose matmuls via PSUM accumulation ----
            gs_tiles = []
            SLOT = N_TABLES * D
            for bt in range(ntiles):
                idx_sb = idx_tiles[bt]
                g = gpool.tile([128, N_TABLES * L * D], BF16, tag="g", name="g")
                nc.gpsimd.indirect_dma_start(
                    out=g[:], out_offset=None,
                    in_=emb[:],
                    in_offset=IndirectOffsetOnAxis(ap=idx_sb[:], axis=0))
                gsum = spool.tile([128, 2 * SLOT], BF16, tag="gsum",
                                  name="gsum")
                gs_tiles.append(gsum)
                gvv = g[:].rearrange("p (a b c) -> p a b c", a=2, b=2)
                sv = gsum[:].rearrange("p (a c) -> p a c", a=2)
                nc.vector.tensor_tensor(sv, gvv[:, :, 0, :], gvv[:, :, 1, :],
                                        op=ADD)

            # ---- main loop: per chunk of 512 samples ----
            for nb in range(nblk):
                h1, h2 = h1b[nb % 2], h2b[nb % 2]
                g1, g2 = g1b[nb % 2], g2b[nb % 2]
                # bottom MLP (feature-major)
                for m in range(4):
                    ps = pmm.tile([128, bw_], F32, tag="ps", name="ps")
                    nc.tensor.matmul(ps[:], lhsT=bw0_sb[:, 128 * m:128 * (m + 1)],
                                     rhs=xt_sb[:, bw_ * nb:bw_ * (nb + 1)],
                                     start=True, stop=True)
                    nc.scalar.activation(h1[m][:], ps[:],
                                         Relu, bias=bb0_sb[:, m:m + 1])
                for m in range(2):
                    ps = pmm.tile([128, bw_], F32, tag="ps", name="ps")
                    for k in range(4):
                        nc.tensor.matmul(
                            ps[:],
                            lhsT=bw1_sb[:, 256 * k + 128 * m:256 * k + 128 * (m + 1)],
                            rhs=h1[k][:],
                            start=(k == 0), stop=(k == 3))
                    nc.scalar.activation(h2[m][:], ps[:],
                                         Relu, bias=bb1_sb[:, m:m + 1])
                x3 = pp.tile([64, bw_], BF16, tag=f"x3_{nb % 2}",
                             name=f"x3_{nb % 2}")
                ps = pmm.tile([64, bw_], F32, tag="ps", name="ps")
                for k in range(2):
                    nc.tensor.matmul(ps[:], lhsT=bw2_sb[:, 64 * k:64 * (k + 1)],
                                     rhs=h2[k][:],
                                     start=(k == 0), stop=(k == 1))
                nc.scalar.activation(x3[:], ps[:], Relu, bias=bb2_sb[:, 0:1])

                zsb = zbpool.tile([64, 13 * bw_], BF16, tag="zsb", name="zsb")
                zsv = zsb[:].rearrange("p (gi s) -> p gi s", gi=13)

                tdms = []
                for bq in range(4):
                    bt = 4 * nb + bq
                    gsum = gs_tiles[bt]  # [128, 2 slot-pairs x 26 x 64]
                    # Tdm [64 d, 32 features x 128 samples] feature-major so
                    # all copy APs have contiguous inner runs; features 27:32
                    # are pads (zeroed once per buffer)
                    tdm = dpool.tile([64, 32 * 128], BF16, tag="tdm",
                                     name="tdm")
                    tdms.append(tdm)
                    tdmF = tdm[:].rearrange("p (f s) -> p f s", s=128)
                    if nb == 0:
                        nc.vector.memset(tdmF[:, NF:32, :], 0.0)
                    # feature 0 = bottom-MLP output
                    nc.gpsimd.tensor_copy(
                        tdmF[:, 0:1, :],
                        x3[:, 128 * bq:128 * (bq + 1)].rearrange(
                            "p (a s) -> p a s", a=1))
                    # features 1..26: two-feature [128,128] PE transposes; the
                    # 2 slot-pairs accumulate in PSUM (rest of the pooling)
                    for w in range(4):
                        p0 = 4 * w
                        npair = min(4, 13 - p0)
                        pst = ptr.tile([128, 512], F32, tag="pst", name="pst")
                        for t in range(npair):
                            blk = 2 * (p0 + t)
                            for sp in range(2):
                                c0 = SLOT * sp + 64 * blk
                                nc.tensor.matmul(
                                    pst[:, 128 * t:128 * (t + 1)],
                                    lhsT=gsum[:, c0:c0 + 128],
                                    rhs=ident[:], start=(sp == 0),
                                    stop=(sp == 1))
                        pstv = pst[:].rearrange("p (t s) -> p t s", s=128)
                        f1 = 2 * p0 + 1
                        nc.vector.tensor_copy(
                            tdmF[0:64, f1:f1 + 2 * npair - 1:2, :],
                            pstv[0:64, 0:npair, :])
                        nc.scalar.copy(
                            tdmF[0:64, f1 + 1:f1 + 2 * npair:2, :],
                            pstv[64:128, 0:npair, :])

                # pass 2: interaction + extraction (separate loop so the
                # in-order PE queue never blocks on a tile's pending fills)
                for bq in range(4):
                    tdm = tdms[bq]
                    # interaction: one [64,32]x[64,26] matmul per sample ->
                    # Z columns land with j uniformly on partitions 0:32;
                    # extraction = 2 batched copies per 16-sample PSUM bank
                    for sw in range(8):
                        zp = pzz.tile([32, 416], F32, tag="zp", name="zp")
                        for sl in range(16):
                            ss = 16 * sw + sl
                            nc.tensor.matmul(
                                zp[:, 26 * sl:26 * (sl + 1)],
                                lhsT=tdm[:, ss:ss + 31 * 128 + 1:128],
                                rhs=tdm[:, 128 + ss:128 + ss + 25 * 128 + 1:128],
                                start=True, stop=True)
                        zpv = zp[:].rearrange("p (s i) -> p i s", i=26)
                        s0 = 128 * bq + 16 * sw
                        for r in range(2):
                            src = zpv[:, r:r + 25:2, :]
                            dst = zsv[32 * r:32 * (r + 1), :, s0:s0 + 16]
                            if r == 0:
                                nc.vector.tensor_copy(dst, src)
                            else:
                                nc.scalar.copy(dst, src)

                # ---- top MLP layer 0: dense-x part + Z via symmetry ----
                for m in range(4):
                    ps = pmm.tile([128, bw_], F32, tag="ps", name="ps")
                    nc.tensor.matmul(ps[:],
                                     lhsT=tw0x_sb[:, 128 * m:128 * (m + 1)],
                                     rhs=x3[:],
                                     start=True, stop=False)
                    for gi in range(13):
                        nc.tensor.matmul(
                            ps[:],
                            lhsT=w0z_sb[:,
                                        512 * gi + 128 * m:
                                        512 * gi + 128 * (m + 1)],
                            rhs=zsb[:, bw_ * gi:bw_ * (gi + 1)],
                            start=False, stop=(gi == 12))
                    nc.scalar.activation(g1[m][:], ps[:],
                                         Relu, bias=tb0_sb[:, m:m + 1])
                # layers 1, 2
                for m in range(2):
                    ps = pmm.tile([128, bw_], F32, tag="ps", name="ps")
                    for k in range(4):
                        nc.tensor.matmul(
                            ps[:],
                            lhsT=tw1_sb[:, 256 * k + 128 * m:
                                        256 * k + 128 * (m + 1)],
                            rhs=g1[k][:],
                            start=(k == 0), stop=(k == 3))
                    nc.scalar.activation(g2[m][:], ps[:],
                                         Relu, bias=tb1_sb[:, m:m + 1])
                zo = pp.tile([1, bw_], F32, tag=f"zo_{nb}", name=f"zo_{nb}")
                ps = pmm.tile([1, bw_], F32, tag="ps", name="ps")
                for k in range(2):
                    nc.tensor.matmul(ps[:], lhsT=tw2_sb[:, k:k + 1],
                                     rhs=g2[k][:],
                                     start=(k == 0), stop=(k == 1))
                nc.scalar.activation(zo[:], ps[:], Sigm, bias=tb2_sb[:, 0:1])
                nc.sync.dma_start(out=out[bw_ * nb:bw_ * (nb + 1)], in_=zo[:])

    nc.compile()
    return nc


def _get_nc():
    global _NC
    if _NC is None:
        _NC = _build_nc()
    return _NC


def kernel(**inputs) -> np.ndarray:
    global LAST_RESULT
    nc = _get_nc()

    emb_bf = np.ascontiguousarray(
        np.asarray(inputs["emb"], dtype=np.float32).reshape(N_TABLES * VOCAB, D)
    ).astype(ml_dtypes.bfloat16)

    dense_x = np.asarray(inputs["dense_x"], dtype=np.float32)
    lS_i = np.asarray(inputs["lS_i"]).reshape(N_TABLES, B, L)
    table_base = np.arange(N_TABLES, dtype=np.int64)[:, None, None] * VOCAB

    def kt(w, p=128):  # [K, M] -> [p, (K//p)*M], k-tiles side by side
        K, M = w.shape
        return np.ascontiguousarray(
            w.reshape(K // p, p, M).transpose(1, 0, 2).reshape(p, -1))

    def bvec(b, p=128):  # [M] -> [p, M//p] (or [M, 1] when M < p)
        M = b.shape[0]
        if M < p:
            return np.ascontiguousarray(b.reshape(M, 1))
        return np.ascontiguousarray(b.reshape(M // p, p).T)

    W = {k: np.asarray(v, dtype=np.float32) for k, v in inputs.items()
         if k.startswith(("bot_", "top_"))}
    t0 = W["top_W0"].T  # [415, 512]

    # w0z[32*r + j, 512*gi + m] = W0 weight of pair (i=2*gi+1+r, j), j < i.
    # Reference pair order: (i, j) for i in range(27) for j in range(i),
    # pair index p = i*(i-1)/2 + j; R = [x (64 cols), Zflat (351)].
    w0z_np = np.zeros((64, 13 * 512), dtype=np.float32)
    zw = t0[64:]  # [351, 512]
    for gi in range(13):
        for r in range(2):
            i = 2 * gi + 1 + r
            base = i * (i - 1) // 2
            w0z_np[32 * r:32 * r + i, 512 * gi:512 * (gi + 1)] = zw[base:base + i]

    shared = {
        "emb": emb_bf,
        "bw0": np.ascontiguousarray(W["bot_W0"].T).astype(ml_dtypes.bfloat16),
        "bw1": kt(W["bot_W1"].T).astype(ml_dtypes.bfloat16),
        "bw2": kt(W["bot_W2"].T).astype(ml_dtypes.bfloat16),
        "bb0": bvec(W["bot_b0"]),
        "bb1": bvec(W["bot_b1"]),
        "bb2": bvec(W["bot_b2"]),
        "tw0x": np.ascontiguousarray(t0[:64]).astype(ml_dtypes.bfloat16),
        "w0z": w0z_np.astype(ml_dtypes.bfloat16),
        "tw1": kt(W["top_W1"].T).astype(ml_dtypes.bfloat16),
        "tw2": kt(W["top_W2"].T).astype(ml_dtypes.bfloat16),
        "tb0": bvec(W["top_b0"]),
        "tb1": bvec(W["top_b1"]),
        "tb2": bvec(W["top_b2"]),
    }

    in_maps = []
    for c in range(N_CORES):
        b0 = c * BL
        # slot-major gather order: idx cols = [slot, table]
        idxc = (table_base + lS_i[:, b0:b0 + BL, :]).transpose(1, 2, 0)
        in_maps.append(dict(
            shared,
            idx=np.ascontiguousarray(idxc.reshape(BL, N_TABLES * L)).astype(np.int32),
            xt=np.ascontiguousarray(dense_x[b0:b0 + BL].T).astype(ml_dtypes.bfloat16),
        ))

    res = run_bass_kernel_spmd(nc, in_maps, core_ids=list(range(N_CORES)),
                               **RUN_KWARGS)
    LAST_RESULT = res
    out = np.concatenate([np.asarray(res.results[c]["out"]) for c in range(N_CORES)])
    return out.reshape(B, 1).astype(np.float32)


# revision 26
# speedup vs baseline: 1.1719x; 1.1078x over previous
"""DLRM forward (nn_DLRM_Net_498216206942) on 8 Trainium2 NeuronCores.

Sharding: data-parallel over the batch — each core takes 2048 of the 16384
samples, with the 26 embedding tables (bf16) and both MLPs replicated.

Per-core kernel layout (v3 — Gram-matmul interaction):
  - Bottom/top MLPs feature-major (features on partitions, batch on free dim).
  - Embedding lookup: one indirect DMA per 128-bag tile gathers all
    26 tables x 4 slots; pooling = 3 DVE/Pool adds (bf16).
  - Pooled features transposed to d-major via 13 two-feature [128,128] PE
    transposes per tile; PSUM halves copied (partition-shifted) into
    Tdm [64 d, 128 samples x 27 features] (feature 0 = bottom-MLP output).
  - Dot interaction: ONE Gram matmul per 4 samples:
    lhsT = rhs = Tdm[:, 108g:108g+108] (K=64, M=113 incl. pad, N=108)
    -> PSUM [113, 108]; diagonal 27x27 blocks are the per-sample Z.
  - Z scattered to zsb [64, 13 x 512] (j on partitions, 2 i-parities) by
    strided PSUM->SBUF copies on Vector/Scalar/Pool engines.
  - Top-MLP layer 0 consumes Z via symmetry: y += w0z_i^T @ zsb blocks,
    accumulated with the dense-x part in one PSUM group.
"""

import sys

sys.path.insert(0, "/opt/trn_rl_repo")

import numpy as np
import ml_dtypes

import concourse.bacc as bacc
import concourse.tile as tile
import concourse.mybir as mybir
from concourse.bass import IndirectOffsetOnAxis
from concourse.bass_utils import run_bass_kernel_spmd
from concourse.masks import make_identity

F32 = mybir.dt.float32
BF16 = mybir.dt.bfloat16
I32 = mybir.dt.int32

N_CORES = 8
N_TABLES = 26
VOCAB = 100000
D = 64
B = 16384
L = 4
BL = B // N_CORES          # 2048 samples per core
NF = N_TABLES + 1          # 27 features in T

_NC = None
LAST_RESULT = None
RUN_KWARGS = {}


def _build_nc():
    ntiles = BL // 128
    V = N_TABLES * VOCAB

    nc = bacc.Bacc("TRN2", target_bir_lowering=False, debug=False,
                   num_devices=N_CORES)

    emb = nc.dram_tensor("emb", [V, D], BF16, kind="ExternalInput")
    idx = nc.dram_tensor("idx", [BL, N_TABLES * L], I32, kind="ExternalInput")
    xt = nc.dram_tensor("xt", [128, BL], BF16, kind="ExternalInput")
    bw0 = nc.dram_tensor("bw0", [128, 512], BF16, kind="ExternalInput")
    bw1 = nc.dram_tensor("bw1", [128, 4 * 256], BF16, kind="ExternalInput")
    bw2 = nc.dram_tensor("bw2", [128, 2 * 64], BF16, kind="ExternalInput")
    bb0 = nc.dram_tensor("bb0", [128, 4], F32, kind="ExternalInput")
    bb1 = nc.dram_tensor("bb1", [128, 2], F32, kind="ExternalInput")
    bb2 = nc.dram_tensor("bb2", [64, 1], F32, kind="ExternalInput")
    # top layer 0: dense-x part [64, 512] bf16 + Z-row weights [64, 13*512]
    tw0x = nc.dram_tensor("tw0x", [128, 512], BF16, kind="ExternalInput")
    w0z = nc.dram_tensor("w0z", [128, 13 * 512], BF16, kind="ExternalInput")
    tw1 = nc.dram_tensor("tw1", [128, 4 * 256], BF16, kind="ExternalInput")
    tw2 = nc.dram_tensor("tw2", [128, 2], BF16, kind="ExternalInput")
    tb0 = nc.dram_tensor("tb0", [128, 4], F32, kind="ExternalInput")
    tb1 = nc.dram_tensor("tb1", [128, 2], F32, kind="ExternalInput")
    tb2 = nc.dram_tensor("tb2", [1, 1], F32, kind="ExternalInput")
    out = nc.dram_tensor("out", [BL], F32, kind="ExternalOutput")

    Relu = mybir.ActivationFunctionType.Relu
    Sigm = mybir.ActivationFunctionType.Sigmoid
    ADD = mybir.AluOpType.add

    bw_ = min(512, BL)
    nblk = BL // bw_

    with tile.TileContext(nc) as tc:
        with (
            tc.tile_pool(name="persist", bufs=1) as pp,
            tc.tile_pool(name="gather", bufs=3) as gpool,
            tc.tile_pool(name="idxp", bufs=6) as ipool,
            tc.tile_pool(name="gsum", bufs=3) as spool,
            tc.tile_pool(name="tdm", bufs=4) as dpool,
            tc.tile_pool(name="zsb", bufs=2) as zbpool,
            tc.tile_pool(name="psum_mm", bufs=3, space="PSUM") as pmm,
            tc.tile_pool(name="psum_tr", bufs=3, space="PSUM") as ptr,
            tc.tile_pool(name="psum_z", bufs=2, space="PSUM") as pzz,
        ):
            idx_tiles = []
            def emit_idx(bt):
                idx_sb = ipool.tile([128, N_TABLES * L], I32, tag="idx_sb",
                                    name="idx_sb")
                idx_tiles.append(idx_sb)
                nc.sync.dma_start(out=idx_sb[:],
                                  in_=idx[128 * bt:128 * (bt + 1), :])
            for bt in range(4):
                emit_idx(bt)

            def load(name, dram, shape, dtype=F32):
                t = pp.tile(shape, dtype, tag=name, name=name)
                nc.sync.dma_start(out=t[:], in_=dram[:])
                return t

            xt_sb = load("xt", xt, [128, BL], BF16)
            bw0_sb = load("bw0", bw0, [128, 512], BF16)
            bw1_sb = load("bw1", bw1, [128, 1024], BF16)
            bw2_sb = load("bw2", bw2, [128, 128], BF16)
            bb0_sb = load("bb0", bb0, [128, 4])
            bb1_sb = load("bb1", bb1, [128, 2])
            bb2_sb = load("bb2", bb2, [64, 1])
            tw0x_sb = load("tw0x", tw0x, [128, 512], BF16)
            w0z_sb = load("w0z", w0z, [128, 13 * 512], BF16)
            tw1_sb = load("tw1", tw1, [128, 1024], BF16)
            tw2_sb = load("tw2", tw2, [128, 2], BF16)
            tb0_sb = load("tb0", tb0, [128, 4])
            tb1_sb = load("tb1", tb1, [128, 2])
            tb2_sb = load("tb2", tb2, [1, 1])

            ident = pp.tile([128, 128], BF16, tag="ident", name="ident")
            make_identity(nc, ident[:])
            for bt in range(4, ntiles):
                emit_idx(bt)

            h1b = [[pp.tile([128, bw_], BF16, tag=f"h1_{m}_{p}",
                            name=f"h1_{m}_{p}") for m in range(4)]
                   for p in range(2)]
            h2b = [[pp.tile([128, bw_], BF16, tag=f"h2_{m}_{p}",
                            name=f"h2_{m}_{p}") for m in range(2)]
                   for p in range(2)]
            g1b = [[pp.tile([128, bw_], BF16, tag=f"g1_{m}_{p}",
                            name=f"g1_{m}_{p}") for m in range(4)]
                   for p in range(2)]
            g2b = [[pp.tile([128, bw_], BF16, tag=f"g2_{m}_{p}",
                            name=f"g2_{m}_{p}") for m in range(2)]
                   for p in range(2)]

            # ---- gather pipeline (gpsimd indirect DMA, slot-major order).
            # One DVE add folds 4 bag slots to 2; the final 2-way sum happens
            # in the transpose matmuls via PSUM accumulation ----
            gs_tiles = []
            SLOT = N_TABLES * D
            for bt in range(ntiles):
                idx_sb = idx_tiles[bt]
                g = gpool.tile([128, N_TABLES * L * D], BF16, tag="g", name="g")
                nc.gpsimd.indirect_dma_start(
                    out=g[:], out_offset=None,
                    in_=emb[:],
                    in_offset=IndirectOffsetOnAxis(ap=idx_sb[:], axis=0))
                gsum = spool.tile([128, 2 * SLOT], BF16, tag="gsum",
                                  name="gsum")
                gs_tiles.append(gsum)
                gvv = g[:].rearrange("p (a b c) -> p a b c", a=2, b=2)
                sv = gsum[:].rearrange("p (a c) -> p a c", a=2)
                nc.vector.tensor_tensor(sv, gvv[:, :, 0, :], gvv[:, :, 1, :],
                                        op=ADD)

            # ---- main loop: per chunk of 512 samples ----
            for nb in range(nblk):
                h1, h2 = h1b[nb % 2], h2b[nb % 2]
                g1, g2 = g1b[nb % 2], g2b[nb % 2]
                # bottom MLP (feature-major)
                for m in range(4):
                    ps = pmm.tile([128, bw_], F32, tag="ps", name="ps")
                    nc.tensor.matmul(ps[:], lhsT=bw0_sb[:, 128 * m:128 * (m + 1)],
                                     rhs=xt_sb[:, bw_ * nb:bw_ * (nb + 1)],
                                     start=True, stop=True)
                    nc.scalar.activation(h1[m][:], ps[:],
                                         Relu, bias=bb0_sb[:, m:m + 1])
                for m in range(2):
                    ps = pmm.tile([128, bw_], F32, tag="ps", name="ps")
                    for k in range(4):
                        nc.tensor.matmul(
                            ps[:],
                            lhsT=bw1_sb[:, 256 * k + 128 * m:256 * k + 128 * (m + 1)],
                            rhs=h1[k][:],
                            start=(k == 0), stop=(k == 3))
                    nc.scalar.activation(h2[m][:], ps[:],
                                         Relu, bias=bb1_sb[:, m:m + 1])
                x3 = pp.tile([128, bw_], BF16, tag=f"x3_{nb % 2}",
                             name=f"x3_{nb % 2}")
                if nb < 2:
                    nc.vector.memset(x3[64:128, :], 0.0)
                ps = pmm.tile([64, bw_], F32, tag="ps", name="ps")
                for k in range(2):
                    nc.tensor.matmul(ps[:], lhsT=bw2_sb[:, 64 * k:64 * (k + 1)],
                                     rhs=h2[k][:],
                                     start=(k == 0), stop=(k == 1))
                nc.scalar.activation(x3[0:64, :], ps[:], Relu,
                                     bias=bb2_sb[:, 0:1])

                zsb = zbpool.tile([128, 13 * bw_], BF16, tag="zsb", name="zsb")
                if nb < 2:
                    nc.vector.memset(zsb[64:128, :], 0.0)
                zsv = zsb[:].rearrange("p (gi s) -> p gi s", gi=13)

                tdms = []
                for bq in range(4):
                    bt = 4 * nb + bq
                    gsum = gs_tiles[bt]  # [128, 2 slot-pairs x 26 x 64]
                    # Tdm [64 d, 32 features x 128 samples] feature-major so
                    # all copy APs have contiguous inner runs; features 27:32
                    # are pads (zeroed once per buffer)
                    tdm = dpool.tile([64, 32 * 128], BF16, tag="tdm",
                                     name="tdm")
                    tdms.append(tdm)
                    tdmF = tdm[:].rearrange("p (f s) -> p f s", s=128)
                    if nb == 0:
                        nc.vector.memset(tdmF[:, NF:32, :], 0.0)
                    # feature 0 = bottom-MLP output
                    nc.gpsimd.tensor_copy(
                        tdmF[:, 0:1, :],
                        x3[0:64, 128 * bq:128 * (bq + 1)].rearrange(
                            "p (a s) -> p a s", a=1))
                    # features 1..26: two-feature [128,128] PE transposes; the
                    # 2 slot-pairs accumulate in PSUM (rest of the pooling)
                    for w in range(4):
                        p0 = 4 * w
                        npair = min(4, 13 - p0)
                        pst = ptr.tile([128, 512], F32, tag="pst", name="pst")
                        for t in range(npair):
                            blk = 2 * (p0 + t)
                            for sp in range(2):
                                c0 = SLOT * sp + 64 * blk
                                nc.tensor.matmul(
                                    pst[:, 128 * t:128 * (t + 1)],
                                    lhsT=gsum[:, c0:c0 + 128],
                                    rhs=ident[:], start=(sp == 0),
                                    stop=(sp == 1))
                        pstv = pst[:].rearrange("p (t s) -> p t s", s=128)
                        f1 = 2 * p0 + 1
                        nc.vector.tensor_copy(
                            tdmF[0:64, f1:f1 + 2 * npair - 1:2, :],
                            pstv[0:64, 0:npair, :])
                        nc.scalar.copy(
                            tdmF[0:64, f1 + 1:f1 + 2 * npair:2, :],
                            pstv[64:128, 0:npair, :])

                # pass 2: interaction + extraction (separate loop so the
                # in-order PE queue never blocks on a tile's pending fills)
                for bq in range(4):
                    tdm = tdms[bq]
                    # interaction: one [64,32]x[64,26] matmul per sample ->
                    # Z columns land with j uniformly on partitions 0:32;
                    # extraction = 2 batched copies per 16-sample PSUM bank
                    for sw in range(8):
                        zp = pzz.tile([32, 416], F32, tag="zp", name="zp")
                        for sl in range(16):
                            ss = 16 * sw + sl
                            nc.tensor.matmul(
                                zp[:, 26 * sl:26 * (sl + 1)],
                                lhsT=tdm[:, ss:ss + 31 * 128 + 1:128],
                                rhs=tdm[:, 128 + ss:128 + ss + 25 * 128 + 1:128],
                                start=True, stop=True)
                        zpv = zp[:].rearrange("p (s i) -> p i s", i=26)
                        s0 = 128 * bq + 16 * sw
                        for r in range(2):
                            src = zpv[:, r:r + 25:2, :]
                            dst = zsv[32 * r:32 * (r + 1), :, s0:s0 + 16]
                            if r == 0:
                                nc.vector.tensor_copy(dst, src)
                            else:
                                nc.scalar.copy(dst, src)

                # ---- top MLP layer 0: dense-x part + Z via symmetry ----
                for m in range(4):
                    ps = pmm.tile([128, bw_], F32, tag="ps", name="ps")
                    nc.tensor.matmul(ps[:],
                                     lhsT=tw0x_sb[:, 128 * m:128 * (m + 1)],
                                     rhs=x3[:],
                                     start=True, stop=False)
                    for gi in range(13):
                        nc.tensor.matmul(
                            ps[:],
                            lhsT=w0z_sb[:,
                                        512 * gi + 128 * m:
                                        512 * gi + 128 * (m + 1)],
                            rhs=zsb[:, bw_ * gi:bw_ * (gi + 1)],
                            start=False, stop=(gi == 12))
                    nc.scalar.activation(g1[m][:], ps[:],
                                         Relu, bias=tb0_sb[:, m:m + 1])
                # layers 1, 2
                for m in range(2):
                    ps = pmm.tile([128, bw_], F32, tag="ps", name="ps")
                    for k in range(4):
                        nc.tensor.matmul(
                            ps[:],
                            lhsT=tw1_sb[:, 256 * k + 128 * m:
                                        256 * k + 128 * (m + 1)],
                            rhs=g1[k][:],
                            start=(k == 0), stop=(k == 3))
                    nc.scalar.activation(g2[m][:], ps[:],
                                         Relu, bias=tb1_sb[:, m:m + 1])
                zo = pp.tile([1, bw_], F32, tag=f"zo_{nb}", name=f"zo_{nb}")
                ps = pmm.tile([1, bw_], F32, tag="ps", name="ps")
                for k in range(2):
                    nc.tensor.matmul(ps[:], lhsT=tw2_sb[:, k:k + 1],
                                     rhs=g2[k][:],
                                     start=(k == 0), stop=(k == 1))
                nc.scalar.activation(zo[:], ps[:], Sigm, bias=tb2_sb[:, 0:1])
                nc.sync.dma_start(out=out[bw_ * nb:bw_ * (nb + 1)], in_=zo[:])

    nc.compile()
    return nc


def _get_nc():
    global _NC
    if _NC is None:
        _NC = _build_nc()
    return _NC


def kernel(**inputs) -> np.ndarray:
    global LAST_RESULT
    nc = _get_nc()

    emb_bf = np.ascontiguousarray(
        np.asarray(inputs["emb"], dtype=np.float32).reshape(N_TABLES * VOCAB, D)
    ).astype(ml_dtypes.bfloat16)

    dense_x = np.asarray(inputs["dense_x"], dtype=np.float32)
    lS_i = np.asarray(inputs["lS_i"]).reshape(N_TABLES, B, L)
    table_base = np.arange(N_TABLES, dtype=np.int64)[:, None, None] * VOCAB

    def kt(w, p=128):  # [K, M] -> [p, (K//p)*M], k-tiles side by side
        K, M = w.shape
        return np.ascontiguousarray(
            w.reshape(K // p, p, M).transpose(1, 0, 2).reshape(p, -1))

    def bvec(b, p=128):  # [M] -> [p, M//p] (or [M, 1] when M < p)
        M = b.shape[0]
        if M < p:
            return np.ascontiguousarray(b.reshape(M, 1))
        return np.ascontiguousarray(b.reshape(M // p, p).T)

    W = {k: np.asarray(v, dtype=np.float32) for k, v in inputs.items()
         if k.startswith(("bot_", "top_"))}
    t0 = W["top_W0"].T  # [415, 512]

    # w0z[32*r + j, 512*gi + m] = W0 weight of pair (i=2*gi+1+r, j), j < i.
    # Rows 64:128 are zero padding so the matmul contracts K=128 (the K=64
    # big-N matmul path runs at half rate on TRN2).
    w0z_np = np.zeros((128, 13 * 512), dtype=np.float32)
    zw = t0[64:]  # [351, 512]
    for gi in range(13):
        for r in range(2):
            i = 2 * gi + 1 + r
            base = i * (i - 1) // 2
            w0z_np[32 * r:32 * r + i, 512 * gi:512 * (gi + 1)] = zw[base:base + i]

    shared = {
        "emb": emb_bf,
        "bw0": np.ascontiguousarray(
            np.concatenate([W["bot_W0"].T,
                            np.zeros((115, 512), np.float32)])
        ).astype(ml_dtypes.bfloat16),
        "bw1": kt(W["bot_W1"].T).astype(ml_dtypes.bfloat16),
        "bw2": kt(W["bot_W2"].T).astype(ml_dtypes.bfloat16),
        "bb0": bvec(W["bot_b0"]),
        "bb1": bvec(W["bot_b1"]),
        "bb2": bvec(W["bot_b2"]),
        "tw0x": np.ascontiguousarray(
            np.concatenate([t0[:64], np.zeros((64, 512), np.float32)])
        ).astype(ml_dtypes.bfloat16),
        "w0z": w0z_np.astype(ml_dtypes.bfloat16),
        "tw1": kt(W["top_W1"].T).astype(ml_dtypes.bfloat16),
        "tw2": kt(W["top_W2"].T).astype(ml_dtypes.bfloat16),
        "tb0": bvec(W["top_b0"]),
        "tb1": bvec(W["top_b1"]),
        "tb2": bvec(W["top_b2"]),
    }

    in_maps = []
    for c in range(N_CORES):
        b0 = c * BL
        # slot-major gather order: idx cols = [slot, table]
        idxc = (table_base + lS_i[:, b0:b0 + BL, :]).transpose(1, 2, 0)
        in_maps.append(dict(
            shared,
            idx=np.ascontiguousarray(idxc.reshape(BL, N_TABLES * L)).astype(np.int32),
            xt=np.ascontiguousarray(
                np.concatenate([dense_x[b0:b0 + BL].T,
                                np.zeros((115, BL), np.float32)])
            ).astype(ml_dtypes.bfloat16),
        ))

    res = run_bass_kernel_spmd(nc, in_maps, core_ids=list(range(N_CORES)),
                               **RUN_KWARGS)
    LAST_RESULT = res
    out = np.concatenate([np.asarray(res.results[c]["out"]) for c in range(N_CORES)])
    return out.reshape(B, 1).astype(np.float32)


# revision 27
# speedup vs baseline: 1.1875x; 1.0133x over previous
"""DLRM forward (nn_DLRM_Net_498216206942) on 8 Trainium2 NeuronCores.

Sharding: data-parallel over the batch — each core takes 2048 of the 16384
samples, with the 26 embedding tables (bf16) and both MLPs replicated.

Per-core kernel layout (v3 — Gram-matmul interaction):
  - Bottom/top MLPs feature-major (features on partitions, batch on free dim).
  - Embedding lookup: one indirect DMA per 128-bag tile gathers all
    26 tables x 4 slots; pooling = 3 DVE/Pool adds (bf16).
  - Pooled features transposed to d-major via 13 two-feature [128,128] PE
    transposes per tile; PSUM halves copied (partition-shifted) into
    Tdm [64 d, 128 samples x 27 features] (feature 0 = bottom-MLP output).
  - Dot interaction: ONE Gram matmul per 4 samples:
    lhsT = rhs = Tdm[:, 108g:108g+108] (K=64, M=113 incl. pad, N=108)
    -> PSUM [113, 108]; diagonal 27x27 blocks are the per-sample Z.
  - Z scattered to zsb [64, 13 x 512] (j on partitions, 2 i-parities) by
    strided PSUM->SBUF copies on Vector/Scalar/Pool engines.
  - Top-MLP layer 0 consumes Z via symmetry: y += w0z_i^T @ zsb blocks,
    accumulated with the dense-x part in one PSUM group.
"""

import sys

sys.path.insert(0, "/opt/trn_rl_repo")

import numpy as np
import ml_dtypes

import concourse.bacc as bacc
import concourse.tile as tile
import concourse.mybir as mybir
from concourse.bass import IndirectOffsetOnAxis
from concourse.bass_utils import run_bass_kernel_spmd
from concourse.masks import make_identity

F32 = mybir.dt.float32
BF16 = mybir.dt.bfloat16
I32 = mybir.dt.int32

N_CORES = 8
N_TABLES = 26
VOCAB = 100000
D = 64
B = 16384
L = 4
BL = B // N_CORES          # 2048 samples per core
NF = N_TABLES + 1          # 27 features in T

_NC = None
LAST_RESULT = None
RUN_KWARGS = {}


def _build_nc():
    ntiles = BL // 128
    V = N_TABLES * VOCAB

    nc = bacc.Bacc("TRN2", target_bir_lowering=False, debug=False,
                   num_devices=N_CORES)

    emb = nc.dram_tensor("emb", [V, D], BF16, kind="ExternalInput")
    idx = nc.dram_tensor("idx", [BL, N_TABLES * L], I32, kind="ExternalInput")
    xt = nc.dram_tensor("xt", [128, BL], BF16, kind="ExternalInput")
    bw0 = nc.dram_tensor("bw0", [128, 512], BF16, kind="ExternalInput")
    bw1 = nc.dram_tensor("bw1", [128, 4 * 256], BF16, kind="ExternalInput")
    bw2 = nc.dram_tensor("bw2", [128, 2 * 64], BF16, kind="ExternalInput")
    bb0 = nc.dram_tensor("bb0", [128, 4], F32, kind="ExternalInput")
    bb1 = nc.dram_tensor("bb1", [128, 2], F32, kind="ExternalInput")
    bb2 = nc.dram_tensor("bb2", [64, 1], F32, kind="ExternalInput")
    # top layer 0: dense-x part [64, 512] bf16 + Z-row weights [64, 13*512]
    tw0x = nc.dram_tensor("tw0x", [128, 512], BF16, kind="ExternalInput")
    w0z = nc.dram_tensor("w0z", [128, 13 * 512], BF16, kind="ExternalInput")
    tw1 = nc.dram_tensor("tw1", [128, 4 * 256], BF16, kind="ExternalInput")
    tw2 = nc.dram_tensor("tw2", [128, 2], BF16, kind="ExternalInput")
    tb0 = nc.dram_tensor("tb0", [128, 4], F32, kind="ExternalInput")
    tb1 = nc.dram_tensor("tb1", [128, 2], F32, kind="ExternalInput")
    tb2 = nc.dram_tensor("tb2", [1, 1], F32, kind="ExternalInput")
    out = nc.dram_tensor("out", [BL], F32, kind="ExternalOutput")

    Relu = mybir.ActivationFunctionType.Relu
    Sigm = mybir.ActivationFunctionType.Sigmoid
    ADD = mybir.AluOpType.add

    bw_ = min(512, BL)
    nblk = BL // bw_

    with tile.TileContext(nc) as tc:
        with (
            tc.tile_pool(name="persist", bufs=1) as pp,
            tc.tile_pool(name="gather", bufs=3) as gpool,
            tc.tile_pool(name="idxp", bufs=6) as ipool,
            tc.tile_pool(name="gsum", bufs=3) as spool,
            tc.tile_pool(name="tdm", bufs=4) as dpool,
            tc.tile_pool(name="zsb", bufs=2) as zbpool,
            tc.tile_pool(name="psum_mm", bufs=3, space="PSUM") as pmm,
            tc.tile_pool(name="psum_tr", bufs=3, space="PSUM") as ptr,
            tc.tile_pool(name="psum_z", bufs=2, space="PSUM") as pzz,
        ):
            idx_tiles = []
            def emit_idx(bt):
                idx_sb = ipool.tile([128, N_TABLES * L], I32, tag="idx_sb",
                                    name="idx_sb")
                idx_tiles.append(idx_sb)
                nc.sync.dma_start(out=idx_sb[:],
                                  in_=idx[128 * bt:128 * (bt + 1), :])
            for bt in range(4):
                emit_idx(bt)

            def load(name, dram, shape, dtype=F32):
                t = pp.tile(shape, dtype, tag=name, name=name)
                nc.sync.dma_start(out=t[:], in_=dram[:])
                return t

            xt_sb = load("xt", xt, [128, BL], BF16)
            bw0_sb = load("bw0", bw0, [128, 512], BF16)
            bw1_sb = load("bw1", bw1, [128, 1024], BF16)
            bw2_sb = load("bw2", bw2, [128, 128], BF16)
            bb0_sb = load("bb0", bb0, [128, 4])
            bb1_sb = load("bb1", bb1, [128, 2])
            bb2_sb = load("bb2", bb2, [64, 1])
            tw0x_sb = load("tw0x", tw0x, [128, 512], BF16)
            w0z_sb = load("w0z", w0z, [128, 13 * 512], BF16)
            tw1_sb = load("tw1", tw1, [128, 1024], BF16)
            tw2_sb = load("tw2", tw2, [128, 2], BF16)
            tb0_sb = load("tb0", tb0, [128, 4])
            tb1_sb = load("tb1", tb1, [128, 2])
            tb2_sb = load("tb2", tb2, [1, 1])

            ident = pp.tile([128, 128], BF16, tag="ident", name="ident")
            make_identity(nc, ident[:])
            for bt in range(4, ntiles):
                emit_idx(bt)

            h1b = [[pp.tile([128, bw_], BF16, tag=f"h1_{m}_{p}",
                            name=f"h1_{m}_{p}") for m in range(4)]
                   for p in range(2)]
            h2b = [[pp.tile([128, bw_], BF16, tag=f"h2_{m}_{p}",
                            name=f"h2_{m}_{p}") for m in range(2)]
                   for p in range(2)]
            g1b = [[pp.tile([128, bw_], BF16, tag=f"g1_{m}_{p}",
                            name=f"g1_{m}_{p}") for m in range(4)]
                   for p in range(2)]
            g2b = [[pp.tile([128, bw_], BF16, tag=f"g2_{m}_{p}",
                            name=f"g2_{m}_{p}") for m in range(2)]
                   for p in range(2)]

            # ---- gather pipeline (gpsimd indirect DMA, slot-major order).
            # One DVE add folds 4 bag slots to 2; the final 2-way sum happens
            # in the transpose matmuls via PSUM accumulation ----
            gs_tiles = []
            SLOT = N_TABLES * D
            for bt in range(ntiles):
                idx_sb = idx_tiles[bt]
                g = gpool.tile([128, N_TABLES * L * D], BF16, tag="g", name="g")
                nc.gpsimd.indirect_dma_start(
                    out=g[:], out_offset=None,
                    in_=emb[:],
                    in_offset=IndirectOffsetOnAxis(ap=idx_sb[:], axis=0))
                g2 = spool.tile([128, 2 * SLOT], BF16, tag="g2", name="g2")
                gvv = g[:].rearrange("p (a b c) -> p a b c", a=2, b=2)
                sv = g2[:].rearrange("p (a c) -> p a c", a=2)
                nc.vector.tensor_tensor(sv, gvv[:, :, 0, :], gvv[:, :, 1, :],
                                        op=ADD)
                gsum = spool.tile([128, SLOT], BF16, tag="gsum", name="gsum")
                gs_tiles.append(gsum)
                nc.vector.tensor_tensor(gsum[:], g2[:, 0:SLOT],
                                        g2[:, SLOT:2 * SLOT], op=ADD)

            # ---- main loop: per chunk of 512 samples ----
            for nb in range(nblk):
                h1, h2 = h1b[nb % 2], h2b[nb % 2]
                g1, g2 = g1b[nb % 2], g2b[nb % 2]
                # bottom MLP (feature-major)
                for m in range(4):
                    ps = pmm.tile([128, bw_], F32, tag="ps", name="ps")
                    nc.tensor.matmul(ps[:], lhsT=bw0_sb[:, 128 * m:128 * (m + 1)],
                                     rhs=xt_sb[:, bw_ * nb:bw_ * (nb + 1)],
                                     start=True, stop=True)
                    nc.scalar.activation(h1[m][:], ps[:],
                                         Relu, bias=bb0_sb[:, m:m + 1])
                for m in range(2):
                    ps = pmm.tile([128, bw_], F32, tag="ps", name="ps")
                    for k in range(4):
                        nc.tensor.matmul(
                            ps[:],
                            lhsT=bw1_sb[:, 256 * k + 128 * m:256 * k + 128 * (m + 1)],
                            rhs=h1[k][:],
                            start=(k == 0), stop=(k == 3))
                    nc.scalar.activation(h2[m][:], ps[:],
                                         Relu, bias=bb1_sb[:, m:m + 1])
                x3 = pp.tile([128, bw_], BF16, tag=f"x3_{nb % 2}",
                             name=f"x3_{nb % 2}")
                if nb < 2:
                    nc.vector.memset(x3[64:128, :], 0.0)
                ps = pmm.tile([64, bw_], F32, tag="ps", name="ps")
                for k in range(2):
                    nc.tensor.matmul(ps[:], lhsT=bw2_sb[:, 64 * k:64 * (k + 1)],
                                     rhs=h2[k][:],
                                     start=(k == 0), stop=(k == 1))
                nc.scalar.activation(x3[0:64, :], ps[:], Relu,
                                     bias=bb2_sb[:, 0:1])

                zsb = zbpool.tile([128, 13 * bw_], BF16, tag="zsb", name="zsb")
                if nb < 2:
                    nc.vector.memset(zsb[64:128, :], 0.0)
                zsv = zsb[:].rearrange("p (gi s) -> p gi s", gi=13)

                tdms = []
                for bq in range(4):
                    bt = 4 * nb + bq
                    gsum = gs_tiles[bt]  # [128, 2 slot-pairs x 26 x 64]
                    # Tdm [64 d, 32 features x 128 samples] feature-major so
                    # all copy APs have contiguous inner runs; features 27:32
                    # are pads (zeroed once per buffer)
                    tdm = dpool.tile([64, 32 * 128], BF16, tag="tdm",
                                     name="tdm")
                    tdms.append(tdm)
                    tdmF = tdm[:].rearrange("p (f s) -> p f s", s=128)
                    if nb == 0:
                        nc.vector.memset(tdmF[:, NF:32, :], 0.0)
                    # feature 0 = bottom-MLP output
                    nc.gpsimd.tensor_copy(
                        tdmF[:, 0:1, :],
                        x3[0:64, 128 * bq:128 * (bq + 1)].rearrange(
                            "p (a s) -> p a s", a=1))
                    # features 1..26: two-feature [128,128] bf16 PE
                    # transposes (is_transpose), 4 per PSUM bank
                    for w in range(4):
                        p0 = 4 * w
                        npair = min(4, 13 - p0)
                        pst = ptr.tile([128, 1024], BF16, tag="pst",
                                       name="pst")
                        for t in range(npair):
                            blk = 2 * (p0 + t)
                            nc.tensor.transpose(
                                pst[:, 128 * t:128 * (t + 1)],
                                gsum[:, 64 * blk:64 * (blk + 2)],
                                ident[:])
                        pstv = pst[:].rearrange("p (t s) -> p t s", s=128)
                        f1 = 2 * p0 + 1
                        nc.vector.tensor_copy(
                            tdmF[0:64, f1:f1 + 2 * npair - 1:2, :],
                            pstv[0:64, 0:npair, :])
                        nc.scalar.copy(
                            tdmF[0:64, f1 + 1:f1 + 2 * npair:2, :],
                            pstv[64:128, 0:npair, :])

                # pass 2: interaction + extraction (separate loop so the
                # in-order PE queue never blocks on a tile's pending fills)
                for bq in range(4):
                    tdm = tdms[bq]
                    # interaction: one [64,32]x[64,26] matmul per sample ->
                    # Z columns land with j uniformly on partitions 0:32;
                    # extraction = 2 batched copies per 16-sample PSUM bank
                    for sw in range(8):
                        zp = pzz.tile([32, 416], F32, tag="zp", name="zp")
                        for sl in range(16):
                            ss = 16 * sw + sl
                            nc.tensor.matmul(
                                zp[:, 26 * sl:26 * (sl + 1)],
                                lhsT=tdm[:, ss:ss + 31 * 128 + 1:128],
                                rhs=tdm[:, 128 + ss:128 + ss + 25 * 128 + 1:128],
                                start=True, stop=True)
                        zpv = zp[:].rearrange("p (s i) -> p i s", i=26)
                        s0 = 128 * bq + 16 * sw
                        for r in range(2):
                            src = zpv[:, r:r + 25:2, :]
                            dst = zsv[32 * r:32 * (r + 1), :, s0:s0 + 16]
                            if r == 0:
                                nc.vector.tensor_copy(dst, src)
                            else:
                                nc.scalar.copy(dst, src)

                # ---- top MLP layer 0: dense-x part + Z via symmetry ----
                for m in range(4):
                    ps = pmm.tile([128, bw_], F32, tag="ps", name="ps")
                    nc.tensor.matmul(ps[:],
                                     lhsT=tw0x_sb[:, 128 * m:128 * (m + 1)],
                                     rhs=x3[:],
                                     start=True, stop=False)
                    for gi in range(13):
                        nc.tensor.matmul(
                            ps[:],
                            lhsT=w0z_sb[:,
                                        512 * gi + 128 * m:
                                        512 * gi + 128 * (m + 1)],
                            rhs=zsb[:, bw_ * gi:bw_ * (gi + 1)],
                            start=False, stop=(gi == 12))
                    nc.scalar.activation(g1[m][:], ps[:],
                                         Relu, bias=tb0_sb[:, m:m + 1])
                # layers 1, 2
                for m in range(2):
                    ps = pmm.tile([128, bw_], F32, tag="ps", name="ps")
                    for k in range(4):
                        nc.tensor.matmul(
                            ps[:],
                            lhsT=tw1_sb[:, 256 * k + 128 * m:
                                        256 * k + 128 * (m + 1)],
                            rhs=g1[k][:],
                            start=(k == 0), stop=(k == 3))
                    nc.scalar.activation(g2[m][:], ps[:],
                                         Relu, bias=tb1_sb[:, m:m + 1])
                zo = pp.tile([1, bw_], F32, tag=f"zo_{nb}", name=f"zo_{nb}")
                ps = pmm.tile([1, bw_], F32, tag="ps", name="ps")
                for k in range(2):
                    nc.tensor.matmul(ps[:], lhsT=tw2_sb[:, k:k + 1],
                                     rhs=g2[k][:],
                                     start=(k == 0), stop=(k == 1))
                nc.scalar.activation(zo[:], ps[:], Sigm, bias=tb2_sb[:, 0:1])
                nc.sync.dma_start(out=out[bw_ * nb:bw_ * (nb + 1)], in_=zo[:])

    nc.compile()
    return nc


def _get_nc():
    global _NC
    if _NC is None:
        _NC = _build_nc()
    return _NC


def kernel(**inputs) -> np.ndarray:
    global LAST_RESULT
    nc = _get_nc()

    emb_bf = np.ascontiguousarray(
        np.asarray(inputs["emb"], dtype=np.float32).reshape(N_TABLES * VOCAB, D)
    ).astype(ml_dtypes.bfloat16)

    dense_x = np.asarray(inputs["dense_x"], dtype=np.float32)
    lS_i = np.asarray(inputs["lS_i"]).reshape(N_TABLES, B, L)
    table_base = np.arange(N_TABLES, dtype=np.int64)[:, None, None] * VOCAB

    def kt(w, p=128):  # [K, M] -> [p, (K//p)*M], k-tiles side by side
        K, M = w.shape
        return np.ascontiguousarray(
            w.reshape(K // p, p, M).transpose(1, 0, 2).reshape(p, -1))

    def bvec(b, p=128):  # [M] -> [p, M//p] (or [M, 1] when M < p)
        M = b.shape[0]
        if M < p:
            return np.ascontiguousarray(b.reshape(M, 1))
        return np.ascontiguousarray(b.reshape(M // p, p).T)

    W = {k: np.asarray(v, dtype=np.float32) for k, v in inputs.items()
         if k.startswith(("bot_", "top_"))}
    t0 = W["top_W0"].T  # [415, 512]

    # w0z[32*r + j, 512*gi + m] = W0 weight of pair (i=2*gi+1+r, j), j < i.
    # Rows 64:128 are zero padding so the matmul contracts K=128 (the K=64
    # big-N matmul path runs at half rate on TRN2).
    w0z_np = np.zeros((128, 13 * 512), dtype=np.float32)
    zw = t0[64:]  # [351, 512]
    for gi in range(13):
        for r in range(2):
            i = 2 * gi + 1 + r
            base = i * (i - 1) // 2
            w0z_np[32 * r:32 * r + i, 512 * gi:512 * (gi + 1)] = zw[base:base + i]

    shared = {
        "emb": emb_bf,
        "bw0": np.ascontiguousarray(
            np.concatenate([W["bot_W0"].T,
                            np.zeros((115, 512), np.float32)])
        ).astype(ml_dtypes.bfloat16),
        "bw1": kt(W["bot_W1"].T).astype(ml_dtypes.bfloat16),
        "bw2": kt(W["bot_W2"].T).astype(ml_dtypes.bfloat16),
        "bb0": bvec(W["bot_b0"]),
        "bb1": bvec(W["bot_b1"]),
        "bb2": bvec(W["bot_b2"]),
        "tw0x": np.ascontiguousarray(
            np.concatenate([t0[:64], np.zeros((64, 512), np.float32)])
        ).astype(ml_dtypes.bfloat16),
        "w0z": w0z_np.astype(ml_dtypes.bfloat16),
        "tw1": kt(W["top_W1"].T).astype(ml_dtypes.bfloat16),
        "tw2": kt(W["top_W2"].T).astype(ml_dtypes.bfloat16),
        "tb0": bvec(W["top_b0"]),
        "tb1": bvec(W["top_b1"]),
        "tb2": bvec(W["top_b2"]),
    }

    in_maps = []
    for c in range(N_CORES):
        b0 = c * BL
        # slot-major gather order: idx cols = [slot, table]
        idxc = (table_base + lS_i[:, b0:b0 + BL, :]).transpose(1, 2, 0)
        in_maps.append(dict(
            shared,
            idx=np.ascontiguousarray(idxc.reshape(BL, N_TABLES * L)).astype(np.int32),
            xt=np.ascontiguousarray(
                np.concatenate([dense_x[b0:b0 + BL].T,
                                np.zeros((115, BL), np.float32)])
            ).astype(ml_dtypes.bfloat16),
        ))

    res = run_bass_kernel_spmd(nc, in_maps, core_ids=list(range(N_CORES)),
                               **RUN_KWARGS)
    LAST_RESULT = res
    out = np.concatenate([np.asarray(res.results[c]["out"]) for c in range(N_CORES)])
    return out.reshape(B, 1).astype(np.float32)


# revision 29
# speedup vs baseline: 1.2731x; 1.0721x over previous
"""DLRM forward (nn_DLRM_Net_498216206942) on 8 Trainium2 NeuronCores.

Sharding: data-parallel over the batch — each core takes 2048 of the 16384
samples, with the 26 embedding tables (bf16) and both MLPs replicated.

Per-core kernel layout (v3 — Gram-matmul interaction):
  - Bottom/top MLPs feature-major (features on partitions, batch on free dim).
  - Embedding lookup: one indirect DMA per 128-bag tile gathers all
    26 tables x 4 slots; pooling = 3 DVE/Pool adds (bf16).
  - Pooled features transposed to d-major via 13 two-feature [128,128] PE
    transposes per tile; PSUM halves copied (partition-shifted) into
    Tdm [64 d, 128 samples x 27 features] (feature 0 = bottom-MLP output).
  - Dot interaction: ONE Gram matmul per 4 samples:
    lhsT = rhs = Tdm[:, 108g:108g+108] (K=64, M=113 incl. pad, N=108)
    -> PSUM [113, 108]; diagonal 27x27 blocks are the per-sample Z.
  - Z scattered to zsb [64, 13 x 512] (j on partitions, 2 i-parities) by
    strided PSUM->SBUF copies on Vector/Scalar/Pool engines.
  - Top-MLP layer 0 consumes Z via symmetry: y += w0z_i^T @ zsb blocks,
    accumulated with the dense-x part in one PSUM group.
"""

import sys

sys.path.insert(0, "/opt/trn_rl_repo")

import numpy as np
import ml_dtypes

import concourse.bacc as bacc
import concourse.tile as tile
import concourse.mybir as mybir
from concourse.bass import IndirectOffsetOnAxis
from concourse.bass_utils import run_bass_kernel_spmd
from concourse.masks import make_identity

F32 = mybir.dt.float32
BF16 = mybir.dt.bfloat16
I32 = mybir.dt.int32

N_CORES = 8
N_TABLES = 26
VOCAB = 100000
D = 64
B = 16384
L = 4
BL = B // N_CORES          # 2048 samples per core
NF = N_TABLES + 1          # 27 features in T

_NC = None
LAST_RESULT = None
RUN_KWARGS = {}


def _build_nc():
    ntiles = BL // 128
    V = N_TABLES * VOCAB

    nc = bacc.Bacc("TRN2", target_bir_lowering=False, debug=False,
                   num_devices=N_CORES)

    emb = nc.dram_tensor("emb", [V, D], BF16, kind="ExternalInput")
    idx = nc.dram_tensor("idx", [BL, N_TABLES * L], I32, kind="ExternalInput")
    xt = nc.dram_tensor("xt", [128, BL], BF16, kind="ExternalInput")
    bw0 = nc.dram_tensor("bw0", [128, 512], BF16, kind="ExternalInput")
    bw1 = nc.dram_tensor("bw1", [128, 4 * 256], BF16, kind="ExternalInput")
    bw2 = nc.dram_tensor("bw2", [128, 2 * 64], BF16, kind="ExternalInput")
    bb0 = nc.dram_tensor("bb0", [128, 4], F32, kind="ExternalInput")
    bb1 = nc.dram_tensor("bb1", [128, 2], F32, kind="ExternalInput")
    bb2 = nc.dram_tensor("bb2", [64, 1], F32, kind="ExternalInput")
    # top layer 0: dense-x part [64, 512] bf16 + Z-row weights [64, 13*512]
    tw0x = nc.dram_tensor("tw0x", [128, 512], BF16, kind="ExternalInput")
    w0z = nc.dram_tensor("w0z", [128, 13 * 512], BF16, kind="ExternalInput")
    tw1 = nc.dram_tensor("tw1", [128, 4 * 256], BF16, kind="ExternalInput")
    tw2 = nc.dram_tensor("tw2", [128, 2], BF16, kind="ExternalInput")
    tb0 = nc.dram_tensor("tb0", [128, 4], F32, kind="ExternalInput")
    tb1 = nc.dram_tensor("tb1", [128, 2], F32, kind="ExternalInput")
    tb2 = nc.dram_tensor("tb2", [1, 1], F32, kind="ExternalInput")
    out = nc.dram_tensor("out", [BL], F32, kind="ExternalOutput")

    Relu = mybir.ActivationFunctionType.Relu
    Sigm = mybir.ActivationFunctionType.Sigmoid
    ADD = mybir.AluOpType.add

    bw_ = min(512, BL)
    nblk = BL // bw_

    with tile.TileContext(nc) as tc:
        with (
            tc.tile_pool(name="persist", bufs=1) as pp,
            tc.tile_pool(name="gather", bufs=3) as gpool,
            tc.tile_pool(name="idxp", bufs=6) as ipool,
            tc.tile_pool(name="gsum", bufs=3) as spool,
            tc.tile_pool(name="tdm", bufs=4) as dpool,
            tc.tile_pool(name="zsb", bufs=2) as zbpool,
            tc.tile_pool(name="psum_mm", bufs=3, space="PSUM") as pmm,
            tc.tile_pool(name="psum_tr", bufs=3, space="PSUM") as ptr,
            tc.tile_pool(name="psum_z", bufs=2, space="PSUM") as pzz,
        ):
            idx_tiles = []
            def emit_idx(bt):
                idx_sb = ipool.tile([128, N_TABLES * L], I32, tag="idx_sb",
                                    name="idx_sb")
                idx_tiles.append(idx_sb)
                nc.sync.dma_start(out=idx_sb[:],
                                  in_=idx[128 * bt:128 * (bt + 1), :])
            # all idx DMAs first so the gathers start immediately; weights
            # after (the big w0z is only needed ~80us in, at top0 of nb 0)
            for bt in range(ntiles):
                emit_idx(bt)

            def load(name, dram, shape, dtype=F32):
                t = pp.tile(shape, dtype, tag=name, name=name)
                nc.sync.dma_start(out=t[:], in_=dram[:])
                return t

            xt_sb = load("xt", xt, [128, BL], BF16)
            bw0_sb = load("bw0", bw0, [128, 512], BF16)
            bw1_sb = load("bw1", bw1, [128, 1024], BF16)
            bw2_sb = load("bw2", bw2, [128, 128], BF16)
            bb0_sb = load("bb0", bb0, [128, 4])
            bb1_sb = load("bb1", bb1, [128, 2])
            bb2_sb = load("bb2", bb2, [64, 1])
            tw0x_sb = load("tw0x", tw0x, [128, 512], BF16)
            tw1_sb = load("tw1", tw1, [128, 1024], BF16)
            tw2_sb = load("tw2", tw2, [128, 2], BF16)
            tb0_sb = load("tb0", tb0, [128, 4])
            tb1_sb = load("tb1", tb1, [128, 2])
            tb2_sb = load("tb2", tb2, [1, 1])
            w0z_sb = load("w0z", w0z, [128, 13 * 512], BF16)

            ident = pp.tile([128, 128], BF16, tag="ident", name="ident")
            make_identity(nc, ident[:])

            h1b = [[pp.tile([128, bw_], BF16, tag=f"h1_{m}_{p}",
                            name=f"h1_{m}_{p}") for m in range(4)]
                   for p in range(2)]
            h2b = [[pp.tile([128, bw_], BF16, tag=f"h2_{m}_{p}",
                            name=f"h2_{m}_{p}") for m in range(2)]
                   for p in range(2)]
            g1b = [[pp.tile([128, bw_], BF16, tag=f"g1_{m}_{p}",
                            name=f"g1_{m}_{p}") for m in range(4)]
                   for p in range(2)]
            g2b = [[pp.tile([128, bw_], BF16, tag=f"g2_{m}_{p}",
                            name=f"g2_{m}_{p}") for m in range(2)]
                   for p in range(2)]

            # ---- gather pipeline (gpsimd indirect DMA, slot-major order).
            # One DVE add folds 4 bag slots to 2; the final 2-way sum happens
            # in the transpose matmuls via PSUM accumulation ----
            gs_tiles = []
            SLOT = N_TABLES * D
            for bt in range(ntiles):
                idx_sb = idx_tiles[bt]
                g = gpool.tile([128, N_TABLES * L * D], BF16, tag="g", name="g")
                nc.gpsimd.indirect_dma_start(
                    out=g[:], out_offset=None,
                    in_=emb[:],
                    in_offset=IndirectOffsetOnAxis(ap=idx_sb[:], axis=0))
                g2 = spool.tile([128, 2 * SLOT], BF16, tag="g2", name="g2")
                gvv = g[:].rearrange("p (a b c) -> p a b c", a=2, b=2)
                sv = g2[:].rearrange("p (a c) -> p a c", a=2)
                nc.vector.tensor_tensor(sv, gvv[:, :, 0, :], gvv[:, :, 1, :],
                                        op=ADD)
                gsum = spool.tile([128, SLOT], BF16, tag="gsum", name="gsum")
                gs_tiles.append(gsum)
                nc.vector.tensor_tensor(gsum[:], g2[:, 0:SLOT],
                                        g2[:, SLOT:2 * SLOT], op=ADD)

            # ---- main loop, software-pipelined at tile granularity:
            # p1(bt+1) [transposes+fills] is emitted before p2(bt)
            # [interaction+extraction] so the in-order PE queue always has
            # independent work while copies drain ----
            x3s, zsbs, zsvs, tdms = {}, {}, {}, {}

            def emit_bottom(nb):
                h1, h2 = h1b[nb % 2], h2b[nb % 2]
                for m in range(4):
                    ps = pmm.tile([128, bw_], F32, tag="ps", name="ps")
                    nc.tensor.matmul(ps[:], lhsT=bw0_sb[:, 128 * m:128 * (m + 1)],
                                     rhs=xt_sb[:, bw_ * nb:bw_ * (nb + 1)],
                                     start=True, stop=True)
                    nc.scalar.activation(h1[m][:], ps[:],
                                         Relu, bias=bb0_sb[:, m:m + 1])
                for m in range(2):
                    ps = pmm.tile([128, bw_], F32, tag="ps", name="ps")
                    for k in range(4):
                        nc.tensor.matmul(
                            ps[:],
                            lhsT=bw1_sb[:, 256 * k + 128 * m:256 * k + 128 * (m + 1)],
                            rhs=h1[k][:],
                            start=(k == 0), stop=(k == 3))
                    nc.scalar.activation(h2[m][:], ps[:],
                                         Relu, bias=bb1_sb[:, m:m + 1])
                x3 = pp.tile([128, bw_], BF16, tag=f"x3_{nb % 2}",
                             name=f"x3_{nb % 2}")
                x3s[nb] = x3
                if nb < 2:
                    nc.vector.memset(x3[64:128, :], 0.0)
                ps = pmm.tile([64, bw_], F32, tag="ps", name="ps")
                for k in range(2):
                    nc.tensor.matmul(ps[:], lhsT=bw2_sb[:, 64 * k:64 * (k + 1)],
                                     rhs=h2[k][:],
                                     start=(k == 0), stop=(k == 1))
                nc.scalar.activation(x3[0:64, :], ps[:], Relu,
                                     bias=bb2_sb[:, 0:1])
                zsb = zbpool.tile([128, 13 * bw_], BF16, tag="zsb", name="zsb")
                zsbs[nb] = zsb
                if nb < 2:
                    nc.vector.memset(zsb[64:128, :], 0.0)
                zsvs[nb] = zsb[:].rearrange("p (gi s) -> p gi s", gi=13)

            def emit_p1(bt):
                nb, bq = bt // 4, bt % 4
                gsum = gs_tiles[bt]
                # Tdm [64 d, 32 features x 128 samples] feature-major so all
                # copy APs have contiguous inner runs; features 27:32 pads
                tdm = dpool.tile([64, 32 * 128], BF16, tag="tdm", name="tdm")
                tdms[bt] = tdm
                tdmF = tdm[:].rearrange("p (f s) -> p f s", s=128)
                if bt < 4:
                    nc.vector.memset(tdmF[:, NF:32, :], 0.0)
                # feature 0 = bottom-MLP output
                nc.gpsimd.tensor_copy(
                    tdmF[:, 0:1, :],
                    x3s[nb][0:64, 128 * bq:128 * (bq + 1)].rearrange(
                        "p (a s) -> p a s", a=1))
                # features 1..26: two-feature [128,128] bf16 PE transposes
                # (is_transpose), 8 per PSUM bank
                for w, (p0, npair) in enumerate(((0, 8), (8, 5))):
                    pst = ptr.tile([128, 128 * npair], BF16, tag="pst",
                                   name="pst")
                    for t in range(npair):
                        blk = 2 * (p0 + t)
                        nc.tensor.transpose(
                            pst[:, 128 * t:128 * (t + 1)],
                            gsum[:, 64 * blk:64 * (blk + 2)],
                            ident[:])
                    pstv = pst[:].rearrange("p (t s) -> p t s", s=128)
                    f1 = 2 * p0 + 1
                    nc.vector.tensor_copy(
                        tdmF[0:64, f1:f1 + 2 * npair - 1:2, :],
                        pstv[0:64, 0:npair, :])
                    nc.scalar.copy(
                        tdmF[0:64, f1 + 1:f1 + 2 * npair:2, :],
                        pstv[64:128, 0:npair, :])

            def emit_p2(bt):
                nb, bq = bt // 4, bt % 4
                tdm, zsv = tdms[bt], zsvs[nb]
                # interaction: one [64,32]x[64,26] matmul per sample ->
                # Z columns land with j uniformly on partitions 0:32;
                # extraction = 2 batched copies per 16-sample PSUM bank
                for sw in range(8):
                    zp = pzz.tile([32, 416], F32, tag="zp", name="zp")
                    for sl in range(16):
                        ss = 16 * sw + sl
                        nc.tensor.matmul(
                            zp[:, 26 * sl:26 * (sl + 1)],
                            lhsT=tdm[:, ss:ss + 31 * 128 + 1:128],
                            rhs=tdm[:, 128 + ss:128 + ss + 25 * 128 + 1:128],
                            start=True, stop=True)
                    zpv = zp[:].rearrange("p (s i) -> p i s", i=26)
                    s0 = 128 * bq + 16 * sw
                    for r in range(2):
                        src = zpv[:, r:r + 25:2, :]
                        dst = zsv[32 * r:32 * (r + 1), :, s0:s0 + 16]
                        if r == 0:
                            nc.vector.tensor_copy(dst, src)
                        else:
                            nc.scalar.copy(dst, src)

            def emit_top(nb):
                g1, g2t = g1b[nb % 2], g2b[nb % 2]
                x3, zsb = x3s[nb], zsbs[nb]
                # top layer 0: dense-x part + Z via symmetry
                for m in range(4):
                    ps = pmm.tile([128, bw_], F32, tag="ps", name="ps")
                    nc.tensor.matmul(ps[:],
                                     lhsT=tw0x_sb[:, 128 * m:128 * (m + 1)],
                                     rhs=x3[:],
                                     start=True, stop=False)
                    for gi in range(13):
                        nc.tensor.matmul(
                            ps[:],
                            lhsT=w0z_sb[:,
                                        512 * gi + 128 * m:
                                        512 * gi + 128 * (m + 1)],
                            rhs=zsb[:, bw_ * gi:bw_ * (gi + 1)],
                            start=False, stop=(gi == 12))
                    nc.scalar.activation(g1[m][:], ps[:],
                                         Relu, bias=tb0_sb[:, m:m + 1])
                for m in range(2):
                    ps = pmm.tile([128, bw_], F32, tag="ps", name="ps")
                    for k in range(4):
                        nc.tensor.matmul(
                            ps[:],
                            lhsT=tw1_sb[:, 256 * k + 128 * m:
                                        256 * k + 128 * (m + 1)],
                            rhs=g1[k][:],
                            start=(k == 0), stop=(k == 3))
                    nc.scalar.activation(g2t[m][:], ps[:],
                                         Relu, bias=tb1_sb[:, m:m + 1])
                zo = pp.tile([1, bw_], F32, tag=f"zo_{nb}", name=f"zo_{nb}")
                ps = pmm.tile([1, bw_], F32, tag="ps", name="ps")
                for k in range(2):
                    nc.tensor.matmul(ps[:], lhsT=tw2_sb[:, k:k + 1],
                                     rhs=g2t[k][:],
                                     start=(k == 0), stop=(k == 1))
                nc.scalar.activation(zo[:], ps[:], Sigm, bias=tb2_sb[:, 0:1])
                nc.sync.dma_start(out=out[bw_ * nb:bw_ * (nb + 1)], in_=zo[:])

            emit_bottom(0)
            emit_p1(0)
            for bt in range(ntiles):
                if bt + 1 < ntiles:
                    if (bt + 1) % 4 == 0:
                        emit_bottom((bt + 1) // 4)
                    emit_p1(bt + 1)
                emit_p2(bt)
                if bt % 4 == 3:
                    emit_top(bt // 4)

    nc.compile()
    return nc


def _get_nc():
    global _NC
    if _NC is None:
        _NC = _build_nc()
    return _NC


def kernel(**inputs) -> np.ndarray:
    global LAST_RESULT
    nc = _get_nc()

    emb_bf = np.ascontiguousarray(
        np.asarray(inputs["emb"], dtype=np.float32).reshape(N_TABLES * VOCAB, D)
    ).astype(ml_dtypes.bfloat16)

    dense_x = np.asarray(inputs["dense_x"], dtype=np.float32)
    lS_i = np.asarray(inputs["lS_i"]).reshape(N_TABLES, B, L)
    table_base = np.arange(N_TABLES, dtype=np.int64)[:, None, None] * VOCAB

    def kt(w, p=128):  # [K, M] -> [p, (K//p)*M], k-tiles side by side
        K, M = w.shape
        return np.ascontiguousarray(
            w.reshape(K // p, p, M).transpose(1, 0, 2).reshape(p, -1))

    def bvec(b, p=128):  # [M] -> [p, M//p] (or [M, 1] when M < p)
        M = b.shape[0]
        if M < p:
            return np.ascontiguousarray(b.reshape(M, 1))
        return np.ascontiguousarray(b.reshape(M // p, p).T)

    W = {k: np.asarray(v, dtype=np.float32) for k, v in inputs.items()
         if k.startswith(("bot_", "top_"))}
    t0 = W["top_W0"].T  # [415, 512]

    # w0z[32*r + j, 512*gi + m] = W0 weight of pair (i=2*gi+1+r, j), j < i.
    # Rows 64:128 are zero padding so the matmul contracts K=128 (the K=64
    # big-N matmul path runs at half rate on TRN2).
    w0z_np = np.zeros((128, 13 * 512), dtype=np.float32)
    zw = t0[64:]  # [351, 512]
    for gi in range(13):
        for r in range(2):
            i = 2 * gi + 1 + r
            base = i * (i - 1) // 2
            w0z_np[32 * r:32 * r + i, 512 * gi:512 * (gi + 1)] = zw[base:base + i]

    shared = {
        "emb": emb_bf,
        "bw0": np.ascontiguousarray(
            np.concatenate([W["bot_W0"].T,
                            np.zeros((115, 512), np.float32)])
        ).astype(ml_dtypes.bfloat16),
        "bw1": kt(W["bot_W1"].T).astype(ml_dtypes.bfloat16),
        "bw2": kt(W["bot_W2"].T).astype(ml_dtypes.bfloat16),
        "bb0": bvec(W["bot_b0"]),
        "bb1": bvec(W["bot_b1"]),
        "bb2": bvec(W["bot_b2"]),
        "tw0x": np.ascontiguousarray(
            np.concatenate([t0[:64], np.zeros((64, 512), np.float32)])
        ).astype(ml_dtypes.bfloat16),
        "w0z": w0z_np.astype(ml_dtypes.bfloat16),
        "tw1": kt(W["top_W1"].T).astype(ml_dtypes.bfloat16),
        "tw2": kt(W["top_W2"].T).astype(ml_dtypes.bfloat16),
        "tb0": bvec(W["top_b0"]),
        "tb1": bvec(W["top_b1"]),
        "tb2": bvec(W["top_b2"]),
    }

    in_maps = []
    for c in range(N_CORES):
        b0 = c * BL
        # slot-major gather order: idx cols = [slot, table]
        idxc = (table_base + lS_i[:, b0:b0 + BL, :]).transpose(1, 2, 0)
        in_maps.append(dict(
            shared,
            idx=np.ascontiguousarray(idxc.reshape(BL, N_TABLES * L)).astype(np.int32),
            xt=np.ascontiguousarray(
                np.concatenate([dense_x[b0:b0 + BL].T,
                                np.zeros((115, BL), np.float32)])
            ).astype(ml_dtypes.bfloat16),
        ))

    res = run_bass_kernel_spmd(nc, in_maps, core_ids=list(range(N_CORES)),
                               **RUN_KWARGS)
    LAST_RESULT = res
    out = np.concatenate([np.asarray(res.results[c]["out"]) for c in range(N_CORES)])
    return out.reshape(B, 1).astype(np.float32)


# revision 30
# speedup vs baseline: 1.2878x; 1.0116x over previous
"""DLRM forward (nn_DLRM_Net_498216206942) on 8 Trainium2 NeuronCores.

Sharding: data-parallel over the batch — each core takes 2048 of the 16384
samples, with the 26 embedding tables (bf16) and both MLPs replicated.

Per-core kernel layout (v3 — Gram-matmul interaction):
  - Bottom/top MLPs feature-major (features on partitions, batch on free dim).
  - Embedding lookup: one indirect DMA per 128-bag tile gathers all
    26 tables x 4 slots; pooling = 3 DVE/Pool adds (bf16).
  - Pooled features transposed to d-major via 13 two-feature [128,128] PE
    transposes per tile; PSUM halves copied (partition-shifted) into
    Tdm [64 d, 128 samples x 27 features] (feature 0 = bottom-MLP output).
  - Dot interaction: ONE Gram matmul per 4 samples:
    lhsT = rhs = Tdm[:, 108g:108g+108] (K=64, M=113 incl. pad, N=108)
    -> PSUM [113, 108]; diagonal 27x27 blocks are the per-sample Z.
  - Z scattered to zsb [64, 13 x 512] (j on partitions, 2 i-parities) by
    strided PSUM->SBUF copies on Vector/Scalar/Pool engines.
  - Top-MLP layer 0 consumes Z via symmetry: y += w0z_i^T @ zsb blocks,
    accumulated with the dense-x part in one PSUM group.
"""

import sys

sys.path.insert(0, "/opt/trn_rl_repo")

import numpy as np
import ml_dtypes

import concourse.bacc as bacc
import concourse.tile as tile
import concourse.mybir as mybir
from concourse.bass import IndirectOffsetOnAxis
from concourse.bass_utils import run_bass_kernel_spmd
from concourse.masks import make_identity

F32 = mybir.dt.float32
BF16 = mybir.dt.bfloat16
I32 = mybir.dt.int32

N_CORES = 8
N_TABLES = 26
VOCAB = 100000
D = 64
B = 16384
L = 4
BL = B // N_CORES          # 2048 samples per core
NF = N_TABLES + 1          # 27 features in T

_NC = None
LAST_RESULT = None
RUN_KWARGS = {}


def _build_nc():
    ntiles = BL // 128
    V = N_TABLES * VOCAB

    nc = bacc.Bacc("TRN2", target_bir_lowering=False, debug=False,
                   num_devices=N_CORES)

    emb = nc.dram_tensor("emb", [V, D], BF16, kind="ExternalInput")
    idx = nc.dram_tensor("idx", [BL, N_TABLES * L], I32, kind="ExternalInput")
    xt = nc.dram_tensor("xt", [128, BL], BF16, kind="ExternalInput")
    bw0 = nc.dram_tensor("bw0", [128, 512], BF16, kind="ExternalInput")
    bw1 = nc.dram_tensor("bw1", [128, 4 * 256], BF16, kind="ExternalInput")
    bw2 = nc.dram_tensor("bw2", [128, 2 * 64], BF16, kind="ExternalInput")
    bb0 = nc.dram_tensor("bb0", [128, 4], F32, kind="ExternalInput")
    bb1 = nc.dram_tensor("bb1", [128, 2], F32, kind="ExternalInput")
    bb2 = nc.dram_tensor("bb2", [64, 1], F32, kind="ExternalInput")
    # top layer 0: dense-x part [64, 512] bf16 + Z-row weights [64, 13*512]
    tw0x = nc.dram_tensor("tw0x", [128, 512], BF16, kind="ExternalInput")
    w0z = nc.dram_tensor("w0z", [128, 13 * 512], BF16, kind="ExternalInput")
    tw1 = nc.dram_tensor("tw1", [128, 4 * 256], BF16, kind="ExternalInput")
    tw2 = nc.dram_tensor("tw2", [128, 2], BF16, kind="ExternalInput")
    tb0 = nc.dram_tensor("tb0", [128, 4], F32, kind="ExternalInput")
    tb1 = nc.dram_tensor("tb1", [128, 2], F32, kind="ExternalInput")
    tb2 = nc.dram_tensor("tb2", [1, 1], F32, kind="ExternalInput")
    out = nc.dram_tensor("out", [BL], F32, kind="ExternalOutput")

    Relu = mybir.ActivationFunctionType.Relu
    Sigm = mybir.ActivationFunctionType.Sigmoid
    ADD = mybir.AluOpType.add

    bw_ = min(512, BL)
    nblk = BL // bw_

    with tile.TileContext(nc) as tc:
        with (
            tc.tile_pool(name="persist", bufs=1) as pp,
            tc.tile_pool(name="gather", bufs=4) as gpool,
            tc.tile_pool(name="idxp", bufs=6) as ipool,
            tc.tile_pool(name="gsum", bufs=3) as spool,
            tc.tile_pool(name="tdm", bufs=4) as dpool,
            tc.tile_pool(name="zsb", bufs=2) as zbpool,
            tc.tile_pool(name="psum_mm", bufs=3, space="PSUM") as pmm,
            tc.tile_pool(name="psum_tr", bufs=3, space="PSUM") as ptr,
            tc.tile_pool(name="psum_z", bufs=2, space="PSUM") as pzz,
        ):
            idx_tiles = []
            def emit_idx(bt):
                idx_sb = ipool.tile([128, N_TABLES * L], I32, tag="idx_sb",
                                    name="idx_sb")
                idx_tiles.append(idx_sb)
                nc.sync.dma_start(out=idx_sb[:],
                                  in_=idx[128 * bt:128 * (bt + 1), :])
            for bt in range(4):
                emit_idx(bt)

            def load(name, dram, shape, dtype=F32):
                t = pp.tile(shape, dtype, tag=name, name=name)
                nc.sync.dma_start(out=t[:], in_=dram[:])
                return t

            xt_sb = load("xt", xt, [128, BL], BF16)
            bw0_sb = load("bw0", bw0, [128, 512], BF16)
            for bt in range(4, ntiles):
                emit_idx(bt)
            bw1_sb = load("bw1", bw1, [128, 1024], BF16)
            bw2_sb = load("bw2", bw2, [128, 128], BF16)
            bb0_sb = load("bb0", bb0, [128, 4])
            bb1_sb = load("bb1", bb1, [128, 2])
            bb2_sb = load("bb2", bb2, [64, 1])
            tw0x_sb = load("tw0x", tw0x, [128, 512], BF16)
            tw1_sb = load("tw1", tw1, [128, 1024], BF16)
            tw2_sb = load("tw2", tw2, [128, 2], BF16)
            tb0_sb = load("tb0", tb0, [128, 4])
            tb1_sb = load("tb1", tb1, [128, 2])
            tb2_sb = load("tb2", tb2, [1, 1])
            w0z_sb = load("w0z", w0z, [128, 13 * 512], BF16)

            ident = pp.tile([128, 128], BF16, tag="ident", name="ident")
            make_identity(nc, ident[:])

            h1b = [[pp.tile([128, bw_], BF16, tag=f"h1_{m}_{p}",
                            name=f"h1_{m}_{p}") for m in range(4)]
                   for p in range(2)]
            h2b = [[pp.tile([128, bw_], BF16, tag=f"h2_{m}_{p}",
                            name=f"h2_{m}_{p}") for m in range(2)]
                   for p in range(2)]
            g1b = [[pp.tile([128, bw_], BF16, tag=f"g1_{m}_{p}",
                            name=f"g1_{m}_{p}") for m in range(4)]
                   for p in range(2)]
            g2b = [[pp.tile([128, bw_], BF16, tag=f"g2_{m}_{p}",
                            name=f"g2_{m}_{p}") for m in range(2)]
                   for p in range(2)]

            # ---- gather pipeline (gpsimd indirect DMA, slot-major order);
            # the pooling adds are emitted per-tile in emit_p1 so the vector
            # queue is never serialized behind later tiles' gathers ----
            g_tiles = []
            SLOT = N_TABLES * D
            for bt in range(ntiles):
                idx_sb = idx_tiles[bt]
                g = gpool.tile([128, N_TABLES * L * D], BF16, tag="g", name="g")
                g_tiles.append(g)
                nc.gpsimd.indirect_dma_start(
                    out=g[:], out_offset=None,
                    in_=emb[:],
                    in_offset=IndirectOffsetOnAxis(ap=idx_sb[:], axis=0))

            # ---- main loop, software-pipelined at tile granularity:
            # p1(bt+1) [transposes+fills] is emitted before p2(bt)
            # [interaction+extraction] so the in-order PE queue always has
            # independent work while copies drain ----
            x3s, zsbs, zsvs, tdms = {}, {}, {}, {}

            def emit_bottom(nb):
                h1, h2 = h1b[nb % 2], h2b[nb % 2]
                for m in range(4):
                    ps = pmm.tile([128, bw_], F32, tag="ps", name="ps")
                    nc.tensor.matmul(ps[:], lhsT=bw0_sb[:, 128 * m:128 * (m + 1)],
                                     rhs=xt_sb[:, bw_ * nb:bw_ * (nb + 1)],
                                     start=True, stop=True)
                    nc.scalar.activation(h1[m][:], ps[:],
                                         Relu, bias=bb0_sb[:, m:m + 1])
                for m in range(2):
                    ps = pmm.tile([128, bw_], F32, tag="ps", name="ps")
                    for k in range(4):
                        nc.tensor.matmul(
                            ps[:],
                            lhsT=bw1_sb[:, 256 * k + 128 * m:256 * k + 128 * (m + 1)],
                            rhs=h1[k][:],
                            start=(k == 0), stop=(k == 3))
                    nc.scalar.activation(h2[m][:], ps[:],
                                         Relu, bias=bb1_sb[:, m:m + 1])
                x3 = pp.tile([128, bw_], BF16, tag=f"x3_{nb % 2}",
                             name=f"x3_{nb % 2}")
                x3s[nb] = x3
                if nb < 2:
                    nc.gpsimd.memset(x3[64:128, :], 0.0)
                ps = pmm.tile([64, bw_], F32, tag="ps", name="ps")
                for k in range(2):
                    nc.tensor.matmul(ps[:], lhsT=bw2_sb[:, 64 * k:64 * (k + 1)],
                                     rhs=h2[k][:],
                                     start=(k == 0), stop=(k == 1))
                nc.scalar.activation(x3[0:64, :], ps[:], Relu,
                                     bias=bb2_sb[:, 0:1])
                zsb = zbpool.tile([128, 13 * bw_], BF16, tag="zsb", name="zsb")
                zsbs[nb] = zsb
                if nb < 2:
                    nc.gpsimd.memset(zsb[64:128, :], 0.0)
                zsvs[nb] = zsb[:].rearrange("p (gi s) -> p gi s", gi=13)

            def emit_p1(bt):
                nb, bq = bt // 4, bt % 4
                g = g_tiles[bt]
                g2 = spool.tile([128, 2 * SLOT], BF16, tag="g2", name="g2")
                gvv = g[:].rearrange("p (a b c) -> p a b c", a=2, b=2)
                sv = g2[:].rearrange("p (a c) -> p a c", a=2)
                nc.vector.tensor_tensor(sv, gvv[:, :, 0, :], gvv[:, :, 1, :],
                                        op=ADD)
                gsum = spool.tile([128, SLOT], BF16, tag="gsum", name="gsum")
                nc.vector.tensor_tensor(gsum[:], g2[:, 0:SLOT],
                                        g2[:, SLOT:2 * SLOT], op=ADD)
                # Tdm [64 d, 32 features x 128 samples] feature-major so all
                # copy APs have contiguous inner runs; features 27:32 pads
                tdm = dpool.tile([64, 32 * 128], BF16, tag="tdm", name="tdm")
                tdms[bt] = tdm
                tdmF = tdm[:].rearrange("p (f s) -> p f s", s=128)
                if bt < 4:
                    nc.gpsimd.memset(tdmF[:, NF:32, :], 0.0)
                # feature 0 = bottom-MLP output
                nc.gpsimd.tensor_copy(
                    tdmF[:, 0:1, :],
                    x3s[nb][0:64, 128 * bq:128 * (bq + 1)].rearrange(
                        "p (a s) -> p a s", a=1))
                # features 1..26: two-feature [128,128] bf16 PE transposes
                # (is_transpose), 8 per PSUM bank
                for w, (p0, npair) in enumerate(((0, 8), (8, 5))):
                    pst = ptr.tile([128, 128 * npair], BF16, tag="pst",
                                   name="pst")
                    for t in range(npair):
                        blk = 2 * (p0 + t)
                        nc.tensor.transpose(
                            pst[:, 128 * t:128 * (t + 1)],
                            gsum[:, 64 * blk:64 * (blk + 2)],
                            ident[:])
                    pstv = pst[:].rearrange("p (t s) -> p t s", s=128)
                    f1 = 2 * p0 + 1
                    nc.vector.tensor_copy(
                        tdmF[0:64, f1:f1 + 2 * npair - 1:2, :],
                        pstv[0:64, 0:npair, :])
                    nc.scalar.copy(
                        tdmF[0:64, f1 + 1:f1 + 2 * npair:2, :],
                        pstv[64:128, 0:npair, :])

            def emit_p2(bt):
                nb, bq = bt // 4, bt % 4
                tdm, zsv = tdms[bt], zsvs[nb]
                # interaction: one [64,32]x[64,26] matmul per sample ->
                # Z columns land with j uniformly on partitions 0:32;
                # extraction = 2 batched copies per 16-sample PSUM bank
                for sw in range(8):
                    zp = pzz.tile([32, 416], F32, tag="zp", name="zp")
                    for sl in range(16):
                        ss = 16 * sw + sl
                        nc.tensor.matmul(
                            zp[:, 26 * sl:26 * (sl + 1)],
                            lhsT=tdm[:, ss:ss + 31 * 128 + 1:128],
                            rhs=tdm[:, 128 + ss:128 + ss + 25 * 128 + 1:128],
                            start=True, stop=True)
                    zpv = zp[:].rearrange("p (s i) -> p i s", i=26)
                    s0 = 128 * bq + 16 * sw
                    for r in range(2):
                        src = zpv[:, r:r + 25:2, :]
                        dst = zsv[32 * r:32 * (r + 1), :, s0:s0 + 16]
                        if r == 0:
                            nc.vector.tensor_copy(dst, src)
                        else:
                            nc.scalar.copy(dst, src)

            def emit_top(nb):
                g1, g2t = g1b[nb % 2], g2b[nb % 2]
                x3, zsb = x3s[nb], zsbs[nb]
                # top layer 0: dense-x part + Z via symmetry
                for m in range(4):
                    ps = pmm.tile([128, bw_], F32, tag="ps", name="ps")
                    nc.tensor.matmul(ps[:],
                                     lhsT=tw0x_sb[:, 128 * m:128 * (m + 1)],
                                     rhs=x3[:],
                                     start=True, stop=False)
                    for gi in range(13):
                        nc.tensor.matmul(
                            ps[:],
                            lhsT=w0z_sb[:,
                                        512 * gi + 128 * m:
                                        512 * gi + 128 * (m + 1)],
                            rhs=zsb[:, bw_ * gi:bw_ * (gi + 1)],
                            start=False, stop=(gi == 12))
                    nc.scalar.activation(g1[m][:], ps[:],
                                         Relu, bias=tb0_sb[:, m:m + 1])
                for m in range(2):
                    ps = pmm.tile([128, bw_], F32, tag="ps", name="ps")
                    for k in range(4):
                        nc.tensor.matmul(
                            ps[:],
                            lhsT=tw1_sb[:, 256 * k + 128 * m:
                                        256 * k + 128 * (m + 1)],
                            rhs=g1[k][:],
                            start=(k == 0), stop=(k == 3))
                    nc.scalar.activation(g2t[m][:], ps[:],
                                         Relu, bias=tb1_sb[:, m:m + 1])
                zo = pp.tile([1, bw_], F32, tag=f"zo_{nb}", name=f"zo_{nb}")
                ps = pmm.tile([1, bw_], F32, tag="ps", name="ps")
                for k in range(2):
                    nc.tensor.matmul(ps[:], lhsT=tw2_sb[:, k:k + 1],
                                     rhs=g2t[k][:],
                                     start=(k == 0), stop=(k == 1))
                nc.scalar.activation(zo[:], ps[:], Sigm, bias=tb2_sb[:, 0:1])
                nc.sync.dma_start(out=out[bw_ * nb:bw_ * (nb + 1)], in_=zo[:])

            emit_bottom(0)
            emit_p1(0)
            for bt in range(ntiles):
                if bt + 1 < ntiles:
                    if (bt + 1) % 4 == 0:
                        emit_bottom((bt + 1) // 4)
                    emit_p1(bt + 1)
                emit_p2(bt)
                if bt % 4 == 3:
                    emit_top(bt // 4)

    nc.compile()
    return nc


def _get_nc():
    global _NC
    if _NC is None:
        _NC = _build_nc()
    return _NC


def kernel(**inputs) -> np.ndarray:
    global LAST_RESULT
    nc = _get_nc()

    emb_bf = np.ascontiguousarray(
        np.asarray(inputs["emb"], dtype=np.float32).reshape(N_TABLES * VOCAB, D)
    ).astype(ml_dtypes.bfloat16)

    dense_x = np.asarray(inputs["dense_x"], dtype=np.float32)
    lS_i = np.asarray(inputs["lS_i"]).reshape(N_TABLES, B, L)
    table_base = np.arange(N_TABLES, dtype=np.int64)[:, None, None] * VOCAB

    def kt(w, p=128):  # [K, M] -> [p, (K//p)*M], k-tiles side by side
        K, M = w.shape
        return np.ascontiguousarray(
            w.reshape(K // p, p, M).transpose(1, 0, 2).reshape(p, -1))

    def bvec(b, p=128):  # [M] -> [p, M//p] (or [M, 1] when M < p)
        M = b.shape[0]
        if M < p:
            return np.ascontiguousarray(b.reshape(M, 1))
        return np.ascontiguousarray(b.reshape(M // p, p).T)

    W = {k: np.asarray(v, dtype=np.float32) for k, v in inputs.items()
         if k.startswith(("bot_", "top_"))}
    t0 = W["top_W0"].T  # [415, 512]

    # w0z[32*r + j, 512*gi + m] = W0 weight of pair (i=2*gi+1+r, j), j < i.
    # Rows 64:128 are zero padding so the matmul contracts K=128 (the K=64
    # big-N matmul path runs at half rate on TRN2).
    w0z_np = np.zeros((128, 13 * 512), dtype=np.float32)
    zw = t0[64:]  # [351, 512]
    for gi in range(13):
        for r in range(2):
            i = 2 * gi + 1 + r
            base = i * (i - 1) // 2
            w0z_np[32 * r:32 * r + i, 512 * gi:512 * (gi + 1)] = zw[base:base + i]

    shared = {
        "emb": emb_bf,
        "bw0": np.ascontiguousarray(
            np.concatenate([W["bot_W0"].T,
                            np.zeros((115, 512), np.float32)])
        ).astype(ml_dtypes.bfloat16),
        "bw1": kt(W["bot_W1"].T).astype(ml_dtypes.bfloat16),
        "bw2": kt(W["bot_W2"].T).astype(ml_dtypes.bfloat16),
        "bb0": bvec(W["bot_b0"]),
        "bb1": bvec(W["bot_b1"]),
        "bb2": bvec(W["bot_b2"]),
        "tw0x": np.ascontiguousarray(
            np.concatenate([t0[:64], np.zeros((64, 512), np.float32)])
        ).astype(ml_dtypes.bfloat16),
        "w0z": w0z_np.astype(ml_dtypes.bfloat16),
        "tw1": kt(W["top_W1"].T).astype(ml_dtypes.bfloat16),
        "tw2": kt(W["top_W2"].T).astype(ml_dtypes.bfloat16),
        "tb0": bvec(W["top_b0"]),
        "tb1": bvec(W["top_b1"]),
        "tb2": bvec(W["top_b2"]),
    }

    in_maps = []
    for c in range(N_CORES):
        b0 = c * BL
        # slot-major gather order: idx cols = [slot, table]
        idxc = (table_base + lS_i[:, b0:b0 + BL, :]).transpose(1, 2, 0)
        in_maps.append(dict(
            shared,
            idx=np.ascontiguousarray(idxc.reshape(BL, N_TABLES * L)).astype(np.int32),
            xt=np.ascontiguousarray(
                np.concatenate([dense_x[b0:b0 + BL].T,
                                np.zeros((115, BL), np.float32)])
            ).astype(ml_dtypes.bfloat16),
        ))

    res = run_bass_kernel_spmd(nc, in_maps, core_ids=list(range(N_CORES)),
                               **RUN_KWARGS)
    LAST_RESULT = res
    out = np.concatenate([np.asarray(res.results[c]["out"]) for c in range(N_CORES)])
    return out.reshape(B, 1).astype(np.float32)


# revision 33
# speedup vs baseline: 1.3035x; 1.0122x over previous
"""DLRM forward (nn_DLRM_Net_498216206942) on 8 Trainium2 NeuronCores.

Sharding: data-parallel over the batch — each core takes 2048 of the 16384
samples, with the 26 embedding tables (bf16) and both MLPs replicated.

Per-core kernel layout (v3 — Gram-matmul interaction):
  - Bottom/top MLPs feature-major (features on partitions, batch on free dim).
  - Embedding lookup: one indirect DMA per 128-bag tile gathers all
    26 tables x 4 slots; pooling = 3 DVE/Pool adds (bf16).
  - Pooled features transposed to d-major via 13 two-feature [128,128] PE
    transposes per tile; PSUM halves copied (partition-shifted) into
    Tdm [64 d, 128 samples x 27 features] (feature 0 = bottom-MLP output).
  - Dot interaction: ONE Gram matmul per 4 samples:
    lhsT = rhs = Tdm[:, 108g:108g+108] (K=64, M=113 incl. pad, N=108)
    -> PSUM [113, 108]; diagonal 27x27 blocks are the per-sample Z.
  - Z scattered to zsb [64, 13 x 512] (j on partitions, 2 i-parities) by
    strided PSUM->SBUF copies on Vector/Scalar/Pool engines.
  - Top-MLP layer 0 consumes Z via symmetry: y += w0z_i^T @ zsb blocks,
    accumulated with the dense-x part in one PSUM group.
"""

import sys

sys.path.insert(0, "/opt/trn_rl_repo")

import numpy as np
import ml_dtypes

import concourse.bacc as bacc
import concourse.tile as tile
import concourse.mybir as mybir
from concourse.bass import IndirectOffsetOnAxis
from concourse.bass_utils import run_bass_kernel_spmd
from concourse.masks import make_identity

F32 = mybir.dt.float32
BF16 = mybir.dt.bfloat16
I32 = mybir.dt.int32

N_CORES = 8
N_TABLES = 26
VOCAB = 100000
D = 64
B = 16384
L = 4
BL = B // N_CORES          # 2048 samples per core
NF = N_TABLES + 1          # 27 features in T

_NC = None
LAST_RESULT = None
RUN_KWARGS = {}


def _build_nc():
    ntiles = BL // 128
    V = N_TABLES * VOCAB

    nc = bacc.Bacc("TRN2", target_bir_lowering=False, debug=False,
                   num_devices=N_CORES)

    emb = nc.dram_tensor("emb", [V, D], BF16, kind="ExternalInput")
    idx = nc.dram_tensor("idx", [BL, N_TABLES * L], I32, kind="ExternalInput")
    xt = nc.dram_tensor("xt", [128, BL], BF16, kind="ExternalInput")
    bw0 = nc.dram_tensor("bw0", [128, 512], BF16, kind="ExternalInput")
    bw1 = nc.dram_tensor("bw1", [128, 4 * 256], BF16, kind="ExternalInput")
    bw2 = nc.dram_tensor("bw2", [128, 2 * 64], BF16, kind="ExternalInput")
    bb0 = nc.dram_tensor("bb0", [128, 4], F32, kind="ExternalInput")
    bb1 = nc.dram_tensor("bb1", [128, 2], F32, kind="ExternalInput")
    bb2 = nc.dram_tensor("bb2", [64, 1], F32, kind="ExternalInput")
    # top layer 0: dense-x part [64, 512] bf16 + Z-row weights [64, 13*512]
    tw0x = nc.dram_tensor("tw0x", [128, 512], BF16, kind="ExternalInput")
    w0z = nc.dram_tensor("w0z", [128, 13 * 512], BF16, kind="ExternalInput")
    tw1 = nc.dram_tensor("tw1", [128, 4 * 256], BF16, kind="ExternalInput")
    tw2 = nc.dram_tensor("tw2", [128, 2], BF16, kind="ExternalInput")
    tb0 = nc.dram_tensor("tb0", [128, 4], F32, kind="ExternalInput")
    tb1 = nc.dram_tensor("tb1", [128, 2], F32, kind="ExternalInput")
    tb2 = nc.dram_tensor("tb2", [1, 1], F32, kind="ExternalInput")
    out = nc.dram_tensor("out", [BL], F32, kind="ExternalOutput")

    Relu = mybir.ActivationFunctionType.Relu
    Sigm = mybir.ActivationFunctionType.Sigmoid
    ADD = mybir.AluOpType.add

    bw_ = min(512, BL)
    nblk = BL // bw_

    with tile.TileContext(nc) as tc:
        with (
            tc.tile_pool(name="persist", bufs=1) as pp,
            tc.tile_pool(name="gather", bufs=4) as gpool,
            tc.tile_pool(name="idxp", bufs=6) as ipool,
            tc.tile_pool(name="gsum", bufs=3) as spool,
            tc.tile_pool(name="tdm", bufs=4) as dpool,
            tc.tile_pool(name="zsb", bufs=2) as zbpool,
            tc.tile_pool(name="psum_mm", bufs=3, space="PSUM") as pmm,
            tc.tile_pool(name="psum_tr", bufs=3, space="PSUM") as ptr,
            tc.tile_pool(name="psum_z", bufs=2, space="PSUM") as pzz,
        ):
            idx_tiles = []
            def emit_idx(bt):
                idx_sb = ipool.tile([128, N_TABLES * L], I32, tag="idx_sb",
                                    name="idx_sb")
                idx_tiles.append(idx_sb)
                nc.sync.dma_start(out=idx_sb[:],
                                  in_=idx[128 * bt:128 * (bt + 1), :])
            for bt in range(4):
                emit_idx(bt)

            def load(name, dram, shape, dtype=F32):
                t = pp.tile(shape, dtype, tag=name, name=name)
                nc.sync.dma_start(out=t[:], in_=dram[:])
                return t

            xt_sb = load("xt", xt, [128, BL], BF16)
            bw0_sb = load("bw0", bw0, [128, 512], BF16)
            for bt in range(4, ntiles):
                emit_idx(bt)
            bw1_sb = load("bw1", bw1, [128, 1024], BF16)
            bw2_sb = load("bw2", bw2, [128, 128], BF16)
            bb0_sb = load("bb0", bb0, [128, 4])
            bb1_sb = load("bb1", bb1, [128, 2])
            bb2_sb = load("bb2", bb2, [64, 1])
            tw0x_sb = load("tw0x", tw0x, [128, 512], BF16)
            tw1_sb = load("tw1", tw1, [128, 1024], BF16)
            tw2_sb = load("tw2", tw2, [128, 2], BF16)
            tb0_sb = load("tb0", tb0, [128, 4])
            tb1_sb = load("tb1", tb1, [128, 2])
            tb2_sb = load("tb2", tb2, [1, 1])
            w0z_sb = load("w0z", w0z, [128, 13 * 512], BF16)

            ident = pp.tile([128, 128], BF16, tag="ident", name="ident")
            make_identity(nc, ident[:])

            h1b = [[pp.tile([128, bw_], BF16, tag=f"h1_{m}_{p}",
                            name=f"h1_{m}_{p}") for m in range(4)]
                   for p in range(2)]
            h2b = [[pp.tile([128, bw_], BF16, tag=f"h2_{m}_{p}",
                            name=f"h2_{m}_{p}") for m in range(2)]
                   for p in range(2)]
            g1b = [[pp.tile([128, bw_], BF16, tag=f"g1_{m}_{p}",
                            name=f"g1_{m}_{p}") for m in range(4)]
                   for p in range(2)]
            g2b = [[pp.tile([128, bw_], BF16, tag=f"g2_{m}_{p}",
                            name=f"g2_{m}_{p}") for m in range(2)]
                   for p in range(2)]

            # ---- gather pipeline (gpsimd indirect DMA, slot-major order);
            # the pooling adds are emitted per-tile in emit_p1 so the vector
            # queue is never serialized behind later tiles' gathers ----
            g_tiles = []
            SLOT = N_TABLES * D
            for bt in range(ntiles):
                idx_sb = idx_tiles[bt]
                g = gpool.tile([128, N_TABLES * L * D], BF16, tag="g", name="g")
                g_tiles.append(g)
                nc.gpsimd.indirect_dma_start(
                    out=g[:], out_offset=None,
                    in_=emb[:],
                    in_offset=IndirectOffsetOnAxis(ap=idx_sb[:], axis=0))

            # ---- main loop, software-pipelined at tile granularity:
            # p1(bt+1) [transposes+fills] is emitted before p2(bt)
            # [interaction+extraction] so the in-order PE queue always has
            # independent work while copies drain ----
            x3s, zsbs, zsvs, tdms = {}, {}, {}, {}

            def emit_bottom(nb):
                h1, h2 = h1b[nb % 2], h2b[nb % 2]
                for m in range(4):
                    ps = pmm.tile([128, bw_], F32, tag="ps", name="ps")
                    nc.tensor.matmul(ps[:], lhsT=bw0_sb[:, 128 * m:128 * (m + 1)],
                                     rhs=xt_sb[:, bw_ * nb:bw_ * (nb + 1)],
                                     start=True, stop=True)
                    nc.scalar.activation(h1[m][:], ps[:],
                                         Relu, bias=bb0_sb[:, m:m + 1])
                for m in range(2):
                    ps = pmm.tile([128, bw_], F32, tag="ps", name="ps")
                    for k in range(4):
                        nc.tensor.matmul(
                            ps[:],
                            lhsT=bw1_sb[:, 256 * k + 128 * m:256 * k + 128 * (m + 1)],
                            rhs=h1[k][:],
                            start=(k == 0), stop=(k == 3))
                    nc.scalar.activation(h2[m][:], ps[:],
                                         Relu, bias=bb1_sb[:, m:m + 1])
                x3 = pp.tile([128, bw_], BF16, tag=f"x3_{nb % 2}",
                             name=f"x3_{nb % 2}")
                x3s[nb] = x3
                if nb < 2:
                    nc.gpsimd.memset(x3[64:128, :], 0.0)
                ps = pmm.tile([64, bw_], F32, tag="ps", name="ps")
                for k in range(2):
                    nc.tensor.matmul(ps[:], lhsT=bw2_sb[:, 64 * k:64 * (k + 1)],
                                     rhs=h2[k][:],
                                     start=(k == 0), stop=(k == 1))
                nc.scalar.activation(x3[0:64, :], ps[:], Relu,
                                     bias=bb2_sb[:, 0:1])
                zsb = zbpool.tile([128, 13 * bw_], BF16, tag="zsb", name="zsb")
                zsbs[nb] = zsb
                if nb < 2:
                    nc.gpsimd.memset(zsb[64:128, :], 0.0)
                zsvs[nb] = zsb[:].rearrange("p (gi s) -> p gi s", gi=13)

            def emit_p1(bt):
                nb, bq = bt // 4, bt % 4
                g = g_tiles[bt]
                g2 = spool.tile([128, 2 * SLOT], BF16, tag="g2", name="g2")
                gvv = g[:].rearrange("p (a b c) -> p a b c", a=2, b=2)
                sv = g2[:].rearrange("p (a c) -> p a c", a=2)
                nc.vector.tensor_tensor(sv, gvv[:, :, 0, :], gvv[:, :, 1, :],
                                        op=ADD)
                gsum = spool.tile([128, SLOT], BF16, tag="gsum", name="gsum")
                nc.vector.tensor_tensor(gsum[:], g2[:, 0:SLOT],
                                        g2[:, SLOT:2 * SLOT], op=ADD)
                # Tdm [64 d, 32 features x 128 samples] feature-major so all
                # copy APs have contiguous inner runs; features 27:32 pads
                tdm = dpool.tile([64, 32 * 128], BF16, tag="tdm", name="tdm")
                tdms[bt] = tdm
                tdmF = tdm[:].rearrange("p (f s) -> p f s", s=128)
                if bt < 4:
                    nc.gpsimd.memset(tdmF[:, NF:32, :], 0.0)
                # feature 0 = bottom-MLP output
                nc.gpsimd.tensor_copy(
                    tdmF[:, 0:1, :],
                    x3s[nb][0:64, 128 * bq:128 * (bq + 1)].rearrange(
                        "p (a s) -> p a s", a=1))
                # features 1..26: two-feature [128,128] bf16 PE transposes
                # (is_transpose), 8 per PSUM bank
                for w, (p0, npair) in enumerate(((0, 8), (8, 5))):
                    pst = ptr.tile([128, 128 * npair], BF16, tag="pst",
                                   name="pst")
                    for t in range(npair):
                        blk = 2 * (p0 + t)
                        nc.tensor.transpose(
                            pst[:, 128 * t:128 * (t + 1)],
                            gsum[:, 64 * blk:64 * (blk + 2)],
                            ident[:])
                    pstv = pst[:].rearrange("p (t s) -> p t s", s=128)
                    f1 = 2 * p0 + 1
                    nc.vector.tensor_copy(
                        tdmF[0:64, f1:f1 + 2 * npair - 1:2, :],
                        pstv[0:64, 0:npair, :])
                    nc.scalar.copy(
                        tdmF[0:64, f1 + 1:f1 + 2 * npair:2, :],
                        pstv[64:128, 0:npair, :])

            def emit_p2(bt):
                nb, bq = bt // 4, bt % 4
                tdm, zsv = tdms[bt], zsvs[nb]
                # interaction: one [64,32]x[64,26] matmul per sample ->
                # Z columns land with j uniformly on partitions 0:32;
                # extraction = 2 batched copies per 16-sample PSUM bank
                for sw in range(8):
                    zp = pzz.tile([32, 416], F32, tag="zp", name="zp")
                    for sl in range(16):
                        ss = 16 * sw + sl
                        nc.tensor.matmul(
                            zp[:, 26 * sl:26 * (sl + 1)],
                            lhsT=tdm[:, ss:ss + 31 * 128 + 1:128],
                            rhs=tdm[:, 128 + ss:128 + ss + 25 * 128 + 1:128],
                            start=True, stop=True)
                    zpv = zp[:].rearrange("p (s i) -> p i s", i=26)
                    s0 = 128 * bq + 16 * sw
                    for r in range(2):
                        src = zpv[:, r:r + 25:2, :]
                        dst = zsv[32 * r:32 * (r + 1), :, s0:s0 + 16]
                        if r == 0:
                            nc.vector.tensor_copy(dst, src)
                        else:
                            nc.scalar.copy(dst, src)

            def emit_top(nb):
                g1, g2t = g1b[nb % 2], g2b[nb % 2]
                x3, zsb = x3s[nb], zsbs[nb]
                # top layer 0: dense-x part + Z via symmetry
                for m in range(4):
                    ps = pmm.tile([128, bw_], F32, tag="ps", name="ps")
                    nc.tensor.matmul(ps[:],
                                     lhsT=tw0x_sb[:, 128 * m:128 * (m + 1)],
                                     rhs=x3[:],
                                     start=True, stop=False)
                    for gi in range(13):
                        nc.tensor.matmul(
                            ps[:],
                            lhsT=w0z_sb[:,
                                        512 * gi + 128 * m:
                                        512 * gi + 128 * (m + 1)],
                            rhs=zsb[:, bw_ * gi:bw_ * (gi + 1)],
                            start=False, stop=(gi == 12))
                    nc.scalar.activation(g1[m][:], ps[:],
                                         Relu, bias=tb0_sb[:, m:m + 1])
                for m in range(2):
                    ps = pmm.tile([128, bw_], F32, tag="ps", name="ps")
                    for k in range(4):
                        nc.tensor.matmul(
                            ps[:],
                            lhsT=tw1_sb[:, 256 * k + 128 * m:
                                        256 * k + 128 * (m + 1)],
                            rhs=g1[k][:],
                            start=(k == 0), stop=(k == 3))
                    nc.scalar.activation(g2t[m][:], ps[:],
                                         Relu, bias=tb1_sb[:, m:m + 1])
                zo = pp.tile([1, bw_], F32, tag=f"zo_{nb}", name=f"zo_{nb}")
                ps = pmm.tile([1, bw_], F32, tag="ps", name="ps")
                for k in range(2):
                    nc.tensor.matmul(ps[:], lhsT=tw2_sb[:, k:k + 1],
                                     rhs=g2t[k][:],
                                     start=(k == 0), stop=(k == 1))
                nc.scalar.activation(zo[:], ps[:], Sigm, bias=tb2_sb[:, 0:1])
                nc.sync.dma_start(out=out[bw_ * nb:bw_ * (nb + 1)], in_=zo[:])

            # prologue: no lookahead for the first two tiles, so their
            # interaction is not queued behind later tiles' gathers
            emit_bottom(0)
            emit_p1(0)
            emit_p2(0)
            emit_p1(1)
            emit_p2(1)
            emit_p1(2)
            for bt in range(2, ntiles):
                if bt + 1 < ntiles:
                    if (bt + 1) % 4 == 0:
                        emit_bottom((bt + 1) // 4)
                    emit_p1(bt + 1)
                emit_p2(bt)
                if bt % 4 == 3:
                    emit_top(bt // 4)

    nc.compile()
    return nc


def _get_nc():
    global _NC
    if _NC is None:
        _NC = _build_nc()
    return _NC


def kernel(**inputs) -> np.ndarray:
    global LAST_RESULT
    nc = _get_nc()

    emb_bf = np.ascontiguousarray(
        np.asarray(inputs["emb"], dtype=np.float32).reshape(N_TABLES * VOCAB, D)
    ).astype(ml_dtypes.bfloat16)

    dense_x = np.asarray(inputs["dense_x"], dtype=np.float32)
    lS_i = np.asarray(inputs["lS_i"]).reshape(N_TABLES, B, L)
    table_base = np.arange(N_TABLES, dtype=np.int64)[:, None, None] * VOCAB

    def kt(w, p=128):  # [K, M] -> [p, (K//p)*M], k-tiles side by side
        K, M = w.shape
        return np.ascontiguousarray(
            w.reshape(K // p, p, M).transpose(1, 0, 2).reshape(p, -1))

    def bvec(b, p=128):  # [M] -> [p, M//p] (or [M, 1] when M < p)
        M = b.shape[0]
        if M < p:
            return np.ascontiguousarray(b.reshape(M, 1))
        return np.ascontiguousarray(b.reshape(M // p, p).T)

    W = {k: np.asarray(v, dtype=np.float32) for k, v in inputs.items()
         if k.startswith(("bot_", "top_"))}
    t0 = W["top_W0"].T  # [415, 512]

    # w0z[32*r + j, 512*gi + m] = W0 weight of pair (i=2*gi+1+r, j), j < i.
    # Rows 64:128 are zero padding so the matmul contracts K=128 (the K=64
    # big-N matmul path runs at half rate on TRN2).
    w0z_np = np.zeros((128, 13 * 512), dtype=np.float32)
    zw = t0[64:]  # [351, 512]
    for gi in range(13):
        for r in range(2):
            i = 2 * gi + 1 + r
            base = i * (i - 1) // 2
            w0z_np[32 * r:32 * r + i, 512 * gi:512 * (gi + 1)] = zw[base:base + i]

    shared = {
        "emb": emb_bf,
        "bw0": np.ascontiguousarray(
            np.concatenate([W["bot_W0"].T,
                            np.zeros((115, 512), np.float32)])
        ).astype(ml_dtypes.bfloat16),
        "bw1": kt(W["bot_W1"].T).astype(ml_dtypes.bfloat16),
        "bw2": kt(W["bot_W2"].T).astype(ml_dtypes.bfloat16),
        "bb0": bvec(W["bot_b0"]),
        "bb1": bvec(W["bot_b1"]),
        "bb2": bvec(W["bot_b2"]),
        "tw0x": np.ascontiguousarray(
            np.concatenate([t0[:64], np.zeros((64, 512), np.float32)])
        ).astype(ml_dtypes.bfloat16),
        "w0z": w0z_np.astype(ml_dtypes.bfloat16),
        "tw1": kt(W["top_W1"].T).astype(ml_dtypes.bfloat16),
        "tw2": kt(W["top_W2"].T).astype(ml_dtypes.bfloat16),
        "tb0": bvec(W["top_b0"]),
        "tb1": bvec(W["top_b1"]),
        "tb2": bvec(W["top_b2"]),
    }

    in_maps = []
    for c in range(N_CORES):
        b0 = c * BL
        # slot-major gather order: idx cols = [slot, table]
        idxc = (table_base + lS_i[:, b0:b0 + BL, :]).transpose(1, 2, 0)
        in_maps.append(dict(
            shared,
            idx=np.ascontiguousarray(idxc.reshape(BL, N_TABLES * L)).astype(np.int32),
            xt=np.ascontiguousarray(
                np.concatenate([dense_x[b0:b0 + BL].T,
                                np.zeros((115, BL), np.float32)])
            ).astype(ml_dtypes.bfloat16),
        ))

    res = run_bass_kernel_spmd(nc, in_maps, core_ids=list(range(N_CORES)),
                               **RUN_KWARGS)
    LAST_RESULT = res
    out = np.concatenate([np.asarray(res.results[c]["out"]) for c in range(N_CORES)])
    return out.reshape(B, 1).astype(np.float32)
